# revision 49
# baseline (speedup 1.0000x reference)
"""Multi-head cross-attention on Trainium2, 8-core SPMD.

Problem (hardcoded): B=4, T=2048, D=1024, H=16 heads, head_dim=64, fp32.
    kv = x_enc @ Wkv + bkv ; q = x_dec @ Wq + bq
    per head: S = q_h k_h^T / sqrt(64); P = softmax(S + mask); O_h = P v_h
    out = concat_h(O_h) @ Wo + bo

Sharding: data parallel over batch (4 slices x 2 cores each) and tensor
parallel over heads within each pair (8 heads per core).  Each core
computes a partial output  Y_local @ Wo[rows_local]  (+bo on the even
core of the pair); the host unshards by summing each pair's partials
and stacking the 4 batch slices.  Host-side shard prep pre-transposes
the activations, regroups Wkv columns, and casts matmul operands to
bf16 (PSUM accumulation stays fp32 on device).

The mask input is structurally zero for this problem (spec fill
"zeros"); softmax(S + 0) == softmax(S), so the kernel does not load it
(checked on the host).

Per-core schedule (bf16 operands, fp32 accumulation):
  A: K^T = Wkv_K^T x_enc^T (head-pair-stacked on partitions); V in
     natural layout directly via x-stationary matmuls, next to a ones
     column that later yields the softmax denominator for free.
     Q-projection for the first q-chunk rides at the end of A.
  C: per q-chunk qc, per head pair, per k-tile: S^T = (K^T tile)^T Q^T
     with zero-padded parity copies of Q (contract over 128, two
     matmuls sharing a stationary); P^T = exp(S^T/8) on ACT; O'^T =
     V_aug^T P^T accumulated in PSUM.  The ACT engine paces this loop,
     so the q-projection of chunk qc+1 and the out-projection of chunk
     qc-1 are chopped into small matmul pieces and interleaved into
     the k-tile stream as PE fillers — PE and ACT both stay ~100%
     busy and no engine idles at chunk boundaries.
"""

import ml_dtypes
import numpy as np

import concourse.bass as bass
import concourse.mybir as mybir
import concourse.tile as tile
from concourse import bacc
from concourse.bass_utils import run_bass_kernel_spmd
from concourse.masks import make_identity

f32 = mybir.dt.float32
bf16 = mybir.dt.bfloat16
np_bf16 = ml_dtypes.bfloat16
AF = mybir.ActivationFunctionType
ALU = mybir.AluOpType

P = 128
MDT = bf16


def build_nc(T=2048, D=1024, HPC=8, HD=64, n_cores=8, debug=False):
    """Build + compile the per-core Bass program. HPC = heads per core."""
    assert HD == 64 and HPC % 2 == 0 and T % 512 == 0 and D % P == 0
    CPC = HPC * HD          # q/out channels per core
    TC = 512                # token chunk (psum free dim), phase A
    QC = 512                # q chunk, attention phase
    NQ = T // TC            # token chunks
    ND = D // P             # model-dim chunks
    NG = HPC // 2           # head pairs
    NKT = T // P            # k-token tiles
    NTT = TC // P           # token tiles per chunk (phase A V)
    HD1 = HD + 1            # V columns + ones column
    SCALE = float(1.0 / np.sqrt(HD))
    ON = min(512, D)        # out-proj free chunk
    NON = D // ON
    NQC = T // QC

    nc = bacc.Bacc("TRN2", target_bir_lowering=False, debug=False,
                   enable_asserts=False, num_devices=n_cores)

    xeT = nc.dram_tensor("x_enc_t", [D, T], MDT, kind="ExternalInput").ap()
    xdT = nc.dram_tensor("x_dec_t", [D, T], MDT, kind="ExternalInput").ap()
    wq_d = nc.dram_tensor("wq", [D, CPC], MDT, kind="ExternalInput").ap()
    wkv_d = nc.dram_tensor("wkv_g", [D, 2 * CPC], MDT, kind="ExternalInput").ap()
    wo_d = nc.dram_tensor("wo", [CPC, D], MDT, kind="ExternalInput").ap()
    bq_d = nc.dram_tensor("bq", [CPC], f32, kind="ExternalInput").ap()
    bkv_d = nc.dram_tensor("bkv_g", [2 * CPC], f32, kind="ExternalInput").ap()
    bo_d = nc.dram_tensor("bo", [D], MDT, kind="ExternalInput").ap()
    # out is stored qt-major: partition p, col (qt*D + c) holds
    # out[qt*128 + p, c] — adjacent qt blocks give 4KB-contiguous
    # per-partition DMA descriptors; the host un-permutes for free
    out_d = nc.dram_tensor("out", [P, (T // P) * D], MDT,
                           kind="ExternalOutput").ap()

    with tile.TileContext(nc) as tc:
      with tc.tile_pool(name="const", bufs=1) as cpool:
        # ---- tiny bias loads first, then wkv, so the first K matmul
        # ---- starts as early as possible
        bo_row = cpool.tile([1, D], MDT, name="bo_row")
        bkv_sb = cpool.tile([P, NG], f32, name="bkv_sb")
        bkv_vrow = cpool.tile([1, CPC], f32, name="bkv_vrow")
        bq_sb = cpool.tile([P, NG], f32, name="bq_sb")
        nc.sync.dma_start(out=bo_row[:], in_=bo_d[:].unsqueeze(0))
        nc.sync.dma_start(out=bkv_vrow[:],
                          in_=bkv_d[CPC:2 * CPC].unsqueeze(0))
        for g in range(NG):
            nc.sync.dma_start(out=bkv_sb[:, g:g + 1],
                              in_=bkv_d[g * P:(g + 1) * P].unsqueeze(1))
            nc.sync.dma_start(out=bq_sb[:, g:g + 1],
                              in_=bq_d[g * P:(g + 1) * P].unsqueeze(1))
        wkv_sb = [cpool.tile([P, 2 * CPC], MDT, name=f"wkv{d}")
                  for d in range(ND)]
        wq_sb = [cpool.tile([P, CPC], MDT, name=f"wq{d}") for d in range(ND)]
        wo_sb = [cpool.tile([P, D], MDT, name=f"wo{g}") for g in range(NG)]

        ident = cpool.tile([P, P], MDT, name="ident")
        make_identity(nc, ident)
        ones_t = cpool.tile([P, P], MDT, name="ones_t")
        nc.vector.tensor_scalar(ones_t[HD:HD + 1, :], ident[HD:HD + 1, :],
                                0.0, 1.0, ALU.mult, ALU.add)
        nc.vector.tensor_scalar(ones_t[0:1, :], ident[0:1, :],
                                0.0, 1.0, ALU.mult, ALU.add)

        # persistent across A->C
        kT = [cpool.tile([P, T], MDT, name=f"kT{g}") for g in range(NG)]
        vnat = [cpool.tile([P, 2 * NKT * HD1], MDT, name=f"vnat{g}")
                for g in range(NG)]     # per pair, head parity h2 in halves
        bo_bc = cpool.tile([P, D], f32, name="bo_bc")
        bkv_vbc = cpool.tile([P, CPC], f32, name="bkv_vbc")

        def vn(h):                      # per-head view [P, NKT*HD1]
            g, h2 = divmod(h, 2)
            off = h2 * NKT * HD1
            return vnat[g][:, off:off + NKT * HD1]

        # ones columns of vnat written once
        for g in range(NG):
            for h2 in range(2):
                blk = vnat[g][:, h2 * NKT * HD1:(h2 + 1) * NKT * HD1] \
                    .rearrange("p (c x) -> p c x", c=NKT)
                nc.vector.tensor_scalar(
                    blk[:, :, HD:HD1], ident[:, 0:NKT].unsqueeze(2),
                    0.0, 1.0, ALU.mult, ALU.add)

        with tc.tile_pool(name="ps_init", bufs=1, space="PSUM") as ips:
            # broadcast bo and the V-part of bkv across partitions via PE
            ps_b = ips.tile([P, D], f32, name="ps_bo")
            for o in range(0, D, 512):
                ow = min(512, D - o)
                nc.tensor.matmul(ps_b[:, o:o + ow], ones_t[0:1, :],
                                 bo_row[0:1, o:o + ow], skip_group_check=True)
            nc.vector.tensor_copy(bo_bc[:], ps_b[:])
            ps_v = ips.tile([P, CPC], f32, name="ps_bkv")
            bkv_vrow16 = cpool.tile([1, CPC], MDT, name="bkv_vrow16")
            nc.vector.tensor_copy(bkv_vrow16[:], bkv_vrow[:])
            nc.tensor.matmul(ps_v[:], ones_t[0:1, :], bkv_vrow16[0:1, :],
                             skip_group_check=True)
            nc.vector.tensor_copy(bkv_vbc[:], ps_v[:])

        # full-row activation tiles (4KB/partition DMA descriptors — the
        # DMA engines are descriptor-rate-bound, so fat rows matter) and
        # padded-parity q tiles live in the const pool
        xeT_sb = [cpool.tile([P, T], MDT, name=f"xeT{d}") for d in range(ND)]
        xdT_sb = [cpool.tile([P, T], MDT, name=f"xdT{d}") for d in range(ND)]
        qTcs = {(qc, g): cpool.tile([P, 2 * QC], MDT, tag=f"qTc{g}", bufs=2,
                                    name=f"qTc_{qc}_{g}")
                for qc in range(NQC) for g in range(NG)}

        def emit_qproj(qc, g, pool, tag):
            """One filler piece: project q-chunk qc, head pair g."""
            qcols = slice(qc * QC, (qc + 1) * QC)
            pq = pool.tile([P, QC], f32, tag=tag, name=f"pq_{qc}_{g}")
            for d in range(ND):
                nc.tensor.matmul(pq[:], wq_sb[d][:, g * P:(g + 1) * P],
                                 xdT_sb[d][:, qcols],
                                 start=(d == 0), stop=(d == ND - 1),
                                 skip_group_check=True)
            qTc = qTcs[(qc, g)]
            nc.vector.tensor_scalar_add(
                qTc[0:HD, 0:QC], pq[0:HD, :], bq_sb[0:HD, g:g + 1])
            nc.vector.tensor_scalar(
                qTc[HD:P, 0:QC], pq[HD:P, :], 0.0, 0.0, ALU.mult, ALU.add)
            nc.vector.tensor_scalar_add(
                qTc[HD:P, QC:2 * QC], pq[HD:P, :], bq_sb[HD:P, g:g + 1])
            nc.vector.tensor_scalar(
                qTc[0:HD, QC:2 * QC], pq[0:HD, :], 0.0, 0.0, ALU.mult, ALU.add)

        # ---------------- Phase A: x_enc^T -> kT, vnat ----------------
        # d-outer accumulation: all 8 PSUM chains (4 K pairs + 4 V token
        # tiles) advance one d-step as soon as wkv[d] + x[d] land, so the
        # PE starts ~1us after the first DMA instead of waiting for the
        # full weight matrix.
        with tc.tile_pool(name="phA_psK", bufs=1, space="PSUM") as apsK, \
             tc.tile_pool(name="phA_psV", bufs=1, space="PSUM") as apsV:
            for tq in range(NQ - 1):     # last chunk rides inside phase C
                tcols = slice(tq * TC, (tq + 1) * TC)
                if tq == 0:
                    # three parallel DGE queues: weights on ACT, x_enc on
                    # SP (first token chunk first so d-step 0 starts after
                    # ~400KB, fat remainder after), x_dec on GPSIMD
                    for d in range(ND):
                        nc.scalar.dma_start(out=wkv_sb[d][:],
                                            in_=wkv_d[d * P:(d + 1) * P, :])
                        nc.sync.dma_start(out=xeT_sb[d][:, 0:TC],
                                          in_=xeT[d * P:(d + 1) * P, 0:TC])
                    for d in range(ND):
                        nc.sync.dma_start(out=xeT_sb[d][:, TC:],
                                          in_=xeT[d * P:(d + 1) * P, TC:])
                        nc.gpsimd.dma_start(out=xdT_sb[d][:],
                                            in_=xdT[d * P:(d + 1) * P, :])
                # later-phase loads ride behind phase-A chunks
                if tq == 1:
                    for d in range(ND):
                        nc.scalar.dma_start(out=wq_sb[d][:],
                                            in_=wq_d[d * P:(d + 1) * P, :])
                if tq == 2:
                    for g in range(NG):
                        nc.scalar.dma_start(out=wo_sb[g][:],
                                            in_=wo_d[g * P:(g + 1) * P, :])
                pks = [apsK.tile([P, TC], f32, tag=f"pk{g}",
                                 name=f"pk_{tq}_{g}") for g in range(NG)]
                pvs = [apsV.tile([P, CPC], f32, tag=f"pv{tt}",
                                 name=f"pv_{tq}_{tt}") for tt in range(NTT)]
                for d in range(ND):
                    for g in range(NG):      # K groups: K^T layout
                        nc.tensor.matmul(pks[g][:],
                                         wkv_sb[d][:, g * P:(g + 1) * P],
                                         xeT_sb[d][:, tcols],
                                         start=(d == 0), stop=(d == ND - 1))
                    for tt in range(NTT):    # V natural: x as stationary
                        nc.tensor.matmul(
                            pvs[tt][:],
                            xeT_sb[d][:, tq * TC + tt * P:
                                        tq * TC + (tt + 1) * P],
                            wkv_sb[d][:, CPC:2 * CPC],
                            start=(d == 0), stop=(d == ND - 1))
                for g in range(NG):
                    nc.vector.tensor_scalar_add(
                        kT[g][:, tcols], pks[g][:], bkv_sb[:, g:g + 1])
                for tt in range(NTT):
                    kt_g = tq * NTT + tt     # global k-token tile index
                    for g in range(NG):      # both parities in one DVE op
                        dst = vnat[g][:, :].rearrange(
                            "p (x c y) -> p x c y", x=2, c=NKT)[:, :, kt_g, 0:HD]
                        src = pvs[tt][:, 2 * g * HD:(2 * g + 2) * HD] \
                            .rearrange("p (x y) -> p x y", x=2)
                        bias = bkv_vbc[:, 2 * g * HD:(2 * g + 2) * HD] \
                            .rearrange("p (x y) -> p x y", x=2)
                        nc.vector.tensor_add(dst, src, bias)
            # q-projection for the first attention chunk's g0/g1 rides
            # here; its g2/g3 ride inside chunk 0 itself
            tcols3 = slice((NQ - 1) * TC, NQ * TC)
            for g in range(2):
                emit_qproj(0, g, apsV, f"pv{g}")

        def tq3_kpiece(g, pool):
            pk = pool.tile([P, TC], f32, tag="pout", name=f"pk3_{g}")
            for d in range(ND):
                nc.tensor.matmul(pk[:], wkv_sb[d][:, g * P:(g + 1) * P],
                                 xeT_sb[d][:, tcols3], start=(d == 0),
                                 stop=(d == ND - 1), skip_group_check=True)
            nc.vector.tensor_scalar_add(kT[g][:, tcols3], pk[:],
                                        bkv_sb[:, g:g + 1])

        def tq3_vpiece(tt, pool):
            kt_g = (NQ - 1) * NTT + tt
            pv = pool.tile([P, CPC], f32, tag="pout", name=f"pv3_{tt}")
            for d in range(ND):
                nc.tensor.matmul(pv[:],
                                 xeT_sb[d][:, (NQ - 1) * TC + tt * P:
                                             (NQ - 1) * TC + (tt + 1) * P],
                                 wkv_sb[d][:, CPC:2 * CPC], start=(d == 0),
                                 stop=(d == ND - 1), skip_group_check=True)
            for g in range(NG):
                dst = vnat[g][:, :].rearrange(
                    "p (x c y) -> p x c y", x=2, c=NKT)[:, :, kt_g, 0:HD]
                src = pv[:, 2 * g * HD:(2 * g + 2) * HD] \
                    .rearrange("p (x y) -> p x y", x=2)
                bias = bkv_vbc[:, 2 * g * HD:(2 * g + 2) * HD] \
                    .rearrange("p (x y) -> p x y", x=2)
                nc.vector.tensor_add(dst, src, bias)

        # ------------- Phase C: attention with interleaved fillers ----------
        with tc.tile_pool(name="phC", bufs=1) as cp2, \
             tc.tile_pool(name="phC_psS", bufs=2, space="PSUM") as psS, \
             tc.tile_pool(name="phC_psO", bufs=2, space="PSUM") as psO, \
             tc.tile_pool(name="phD_ps", bufs=2, space="PSUM") as psD:

            yTss = {}
            ost_hold = {}

            def outproj_piece(qc, qt):
                """Filler piece: one 128-row block of out = Y @ Wo + bo.
                Two qt blocks share one ost tile and leave in one fat DMA."""
                yTs = yTss[qc]
                if qt % 2 == 0:
                    ost_hold[qc] = cp2.tile([P, 2 * D], MDT, tag="ost",
                                            bufs=2, name=f"ost_{qc}_{qt}")
                ost = ost_hold[qc]
                ob = (qt % 2) * D
                for o in range(NON):
                    ocols = slice(o * ON, (o + 1) * ON)
                    pout = psD.tile([P, ON], f32, tag="pout",
                                    name=f"pout_{qc}_{qt}_{o}")
                    for g in range(NG):
                        nc.tensor.matmul(
                            pout[:], yTs[g][:, qt * P:(qt + 1) * P],
                            wo_sb[g][:, ocols],
                            start=(g == 0), stop=(g == NG - 1),
                            skip_group_check=True)
                    nc.vector.tensor_add(ost[:, ob + o * ON:ob + (o + 1) * ON],
                                         pout[:], bo_bc[:, ocols])
                if qt % 2 == 1:
                    qtg = qc * (QC // P) + qt - 1
                    nc.sync.dma_start(
                        out=out_d[:, qtg * D:(qtg + 2) * D], in_=ost[:])

            def qp_piece(qc, g):
                return lambda: emit_qproj(qc, g, psD, "pout")

            def op_piece(qc, qt):
                return lambda: outproj_piece(qc, qt)

            for qc in range(NQC):
                # filler pieces for this chunk's k-tile stream; the last
                # K/V chunk's projections go into chunk 0 as per-head-pair
                # priority pieces (deadline: that pair's kt=12 matmul);
                # out-projections are spread over later chunks so every
                # chunk's filler supply matches its exp-pacing deficit
                gprio = {}
                if qc == 0:
                    # vpieces feed every pair's kt=12..15 -> all inside g0;
                    # kpiece g feeds only pair g's kt=12..15 -> one per g
                    gprio[0] = [lambda tt=tt: tq3_vpiece(tt, psD)
                                for tt in range(NTT)] \
                        + [lambda: tq3_kpiece(0, psD)]
                    gprio[1] = [lambda: tq3_kpiece(1, psD), qp_piece(0, 2)]
                    gprio[2] = [lambda: tq3_kpiece(2, psD), qp_piece(0, 3)]
                    gprio[3] = [lambda: tq3_kpiece(3, psD)]
                    rest = [qp_piece(1, 0), qp_piece(1, 1)]
                else:
                    # this chunk's own g2/g3 q-projections ride in its g0/g1
                    gprio[0] = [qp_piece(qc, 2)]
                    gprio[1] = [qp_piece(qc, 3)]
                    if qc == 1:
                        rest = [qp_piece(2, 0), qp_piece(2, 1),
                                op_piece(0, 0), op_piece(0, 1)]
                    elif qc == 2:
                        rest = [qp_piece(3, 0), qp_piece(3, 1),
                                op_piece(0, 2), op_piece(0, 3),
                                op_piece(1, 0)]
                    else:
                        rest = [op_piece(1, 1), op_piece(1, 2),
                                op_piece(1, 3)] \
                            + [op_piece(2, qt) for qt in range(QC // P)]
                # last chunk: front-load so the tail past the final exp
                # holds only the unavoidable last out-projection
                nslots = 24 if qc == NQC - 1 else NG * (NKT // 2)
                nrest = len(rest)
                rpopped = 0
                slot = 0
                cur_prio = []

                def tick():
                    nonlocal slot, rpopped
                    slot += 1
                    if cur_prio:
                        cur_prio.pop(0)()
                        return
                    while rpopped < min(nrest, nrest * slot // nslots):
                        rest[rpopped]()
                        rpopped += 1

                yTs = [cp2.tile([P, QC], MDT, tag=f"yT{g}", bufs=3,
                                name=f"yT_{qc}_{g}") for g in range(NG)]
                yTss[qc] = yTs
                for g in range(NG):
                    cur_prio = gprio.get(g, [])
                    po = [psO.tile([P, QC], f32, tag="po",
                                   name=f"po_{qc}_{g}_{h2}")
                          for h2 in range(2)]
                    pss = {}
                    pts = {}
                    qTc = qTcs[(qc, g)]

                    def mm1(kt, qc=qc, g=g, qTc=qTc, pss=pss):
                        ps = psS.tile([P, 2 * QC], f32, tag="ps",
                                      name=f"ps_{qc}_{g}_{kt}")
                        for h2 in range(2):
                            nc.tensor.matmul(
                                ps[:, h2 * QC:(h2 + 1) * QC],
                                kT[g][:, kt * P:(kt + 1) * P],
                                qTc[:, h2 * QC:(h2 + 1) * QC],
                                skip_group_check=True)
                        pss[kt] = ps

                    def do_exp(kt, qc=qc, g=g, pss=pss, pts=pts):
                        pt = cp2.tile([P, 2 * QC], MDT, tag="pt", bufs=4,
                                      name=f"pt_{qc}_{g}_{kt}")
                        nc.scalar.activation(pt[:], pss[kt][:], AF.Exp,
                                             scale=SCALE)
                        pts[kt] = pt

                    def mm2(kt, g=g, po=po, pts=pts):
                        pt = pts[kt]
                        for h2 in range(2):
                            nc.tensor.matmul(
                                po[h2][0:HD1, :],
                                vn(2 * g + h2)[:, kt * HD1:(kt + 1) * HD1],
                                pt[:, h2 * QC:(h2 + 1) * QC],
                                start=(kt == 0), stop=(kt == NKT - 1),
                                skip_group_check=True)

                    # software pipeline over kt with PE fillers
                    mm1(0)
                    mm1(1)
                    do_exp(0)
                    do_exp(1)
                    for k2 in range(1, NKT // 2):
                        mm1(2 * k2)
                        mm1(2 * k2 + 1)
                        mm2(2 * k2 - 2)
                        mm2(2 * k2 - 1)
                        do_exp(2 * k2)
                        do_exp(2 * k2 + 1)
                        tick()
                    mm2(NKT - 2)
                    mm2(NKT - 1)
                    tick()

                    # normalize + evict to yT
                    srow = [cp2.tile([P, QC], MDT, tag="srow", bufs=2,
                                     name=f"srow_{qc}_{g}_{h2}")
                            for h2 in range(2)]
                    rbc = [cp2.tile([P, QC], f32, tag="rbc", bufs=2,
                                    name=f"rbc_{qc}_{g}_{h2}")
                           for h2 in range(2)]
                    scr = [cp2.tile([P, QC], f32, tag="scr", bufs=2,
                                    name=f"scr_{qc}_{g}_{h2}")
                           for h2 in range(2)]
                    ps_bc = [psD.tile([P, QC], f32, tag="pout",
                                      name=f"psbc_{qc}_{g}_{h2}")
                             for h2 in range(2)]
                    for h2 in range(2):
                        nc.vector.tensor_copy(srow[h2][HD:HD1, :],
                                              po[h2][HD:HD1, :])
                    for h2 in range(2):
                        nc.tensor.matmul(ps_bc[h2][0:HD, :],
                                         ones_t[HD:HD1, 0:HD],
                                         srow[h2][HD:HD1, :],
                                         skip_group_check=True)
                    for h2 in range(2):
                        nc.vector.reciprocal_approx_accurate(
                            out=rbc[h2][0:HD, :], in_=ps_bc[h2][0:HD, :],
                            scratch=scr[h2][0:HD, :])
                    nc.vector.tensor_mul(yTs[g][0:HD, :],
                                         po[0][0:HD, :], rbc[0][0:HD, :])
                    stg = cp2.tile([P, QC], MDT, tag="stg",
                                   bufs=2, name=f"stg_{qc}_{g}")
                    nc.vector.tensor_mul(stg[0:HD, :],
                                         po[1][0:HD, :], rbc[1][0:HD, :])
                    nc.sync.dma_start(out=yTs[g][HD:P, :],
                                      in_=stg[0:HD, :])
                # leftovers (shouldn't happen, but keep it correct)
                for g in range(NG):
                    while gprio.get(g):
                        gprio[g].pop(0)()
                while rpopped < nrest:
                    rest[rpopped]()
                    rpopped += 1

            # tail: out-projection of the last chunk
            for qt in range(QC // P):
                outproj_piece(NQC - 1, qt)

    nc.compile()
    return nc


# ---------------------------------------------------------------------------
# Host side: sharding, run, unshard
# ---------------------------------------------------------------------------

_NC_CACHE = {}


def _get_nc():
    key = "full"
    if key not in _NC_CACHE:
        _NC_CACHE[key] = build_nc()
    return _NC_CACHE[key]


def _group_kv_cols(w_slice, HPC, HD):
    """Reorder kv columns [h-major, (k|v), d] -> K head-pair groups then V."""
    last = w_slice.shape[-1]
    assert last == HPC * 2 * HD
    arr = w_slice.reshape(w_slice.shape[:-1] + (HPC, 2, HD))
    kpart = arr[..., :, 0, :].reshape(w_slice.shape[:-1] + (HPC * HD,))
    vpart = arr[..., :, 1, :].reshape(w_slice.shape[:-1] + (HPC * HD,))
    return np.ascontiguousarray(np.concatenate([kpart, vpart], axis=-1))


def make_in_maps(x_enc, x_dec, Wq, bq, Wkv, bkv, Wo, bo, n_cores=8,
                 HPC=8, HD=64):
    CPC = HPC * HD
    mdt = np_bf16
    in_maps = []
    xet = [np.ascontiguousarray(x_enc[b].T).astype(mdt)
           for b in range(x_enc.shape[0])]
    xdt = [np.ascontiguousarray(x_dec[b].T).astype(mdt)
           for b in range(x_dec.shape[0])]
    for c in range(n_cores):
        b, hg = c // 2, c % 2
        wkv_slice = Wkv[:, hg * 2 * CPC:(hg + 1) * 2 * CPC]
        bkv_slice = bkv[hg * 2 * CPC:(hg + 1) * 2 * CPC]
        in_maps.append({
            "x_enc_t": xet[b],
            "x_dec_t": xdt[b],
            "wq": np.ascontiguousarray(Wq[:, hg * CPC:(hg + 1) * CPC]).astype(mdt),
            "wkv_g": _group_kv_cols(wkv_slice, HPC, HD).astype(mdt),
            "wo": np.ascontiguousarray(Wo[hg * CPC:(hg + 1) * CPC, :]).astype(mdt),
            "bq": np.ascontiguousarray(bq[hg * CPC:(hg + 1) * CPC]),
            "bkv_g": _group_kv_cols(bkv_slice, HPC, HD),
            "bo": (np.ascontiguousarray(bo) if hg == 0
                   else np.zeros_like(bo)).astype(mdt),
        })
    return in_maps


def unshard(outs, B=4, T=2048, D=1024):
    def unpack(a):
        # device layout: [128, (T//128)*D] qt-major -> [T, D]
        return np.ascontiguousarray(
            a.astype(np.float32).reshape(P, T // P, D).transpose(1, 0, 2)
            .reshape(T, D))
    return np.stack([unpack(outs[2 * b]) + unpack(outs[2 * b + 1])
                     for b in range(B)], axis=0)


def kernel(x_enc, x_dec, mask, Wq, bq, Wkv, bkv, Wo, bo):
    x_enc = np.asarray(x_enc, dtype=np.float32)
    x_dec = np.asarray(x_dec, dtype=np.float32)
    Wq = np.asarray(Wq, dtype=np.float32)
    bq = np.asarray(bq, dtype=np.float32)
    Wkv = np.asarray(Wkv, dtype=np.float32)
    bkv = np.asarray(bkv, dtype=np.float32)
    Wo = np.asarray(Wo, dtype=np.float32)
    bo = np.asarray(bo, dtype=np.float32)
    mask = np.asarray(mask)
    if mask.any():
        raise ValueError("kernel assumes a zero additive mask (spec fill=zeros)")

    nc = _get_nc()
    in_maps = make_in_maps(x_enc, x_dec, Wq, bq, Wkv, bkv, Wo, bo)
    res = run_bass_kernel_spmd(nc, in_maps, core_ids=list(range(8)))
    outs = [res.results[c]["out"] for c in range(8)]
    return unshard(outs, B=x_enc.shape[0])


if __name__ == "__main__":
    import time
    t0 = time.time()
    nc = _get_nc()
    print(f"build+compile ok in {time.time() - t0:.1f}s")


# revision 54
# speedup vs baseline: 1.0334x; 1.0334x over previous
"""Multi-head cross-attention on Trainium2, 8-core SPMD.

Problem (hardcoded): B=4, T=2048, D=1024, H=16 heads, head_dim=64, fp32.
    kv = x_enc @ Wkv + bkv ; q = x_dec @ Wq + bq
    per head: S = q_h k_h^T / sqrt(64); P = softmax(S + mask); O_h = P v_h
    out = concat_h(O_h) @ Wo + bo

Sharding: data parallel over batch (4 slices x 2 cores each) and tensor
parallel over heads within each pair (8 heads per core).  Each core
computes a partial output  Y_local @ Wo[rows_local]  (+bo on the even
core of the pair); the host unshards by summing each pair's partials
and stacking the 4 batch slices.  Host-side shard prep pre-transposes
the activations, regroups Wkv columns, and casts matmul operands to
bf16 (PSUM accumulation stays fp32 on device).

The mask input is structurally zero for this problem (spec fill
"zeros"); softmax(S + 0) == softmax(S), so the kernel does not load it
(checked on the host).

Per-core schedule (bf16 operands, fp32 accumulation):
  A: K^T = Wkv_K^T x_enc^T (head-pair-stacked on partitions); V in
     natural layout directly via x-stationary matmuls, next to a ones
     column that later yields the softmax denominator for free.
     Q-projection for the first q-chunk rides at the end of A.
  C: per q-chunk qc, per head pair, per k-tile: S^T = (K^T tile)^T Q^T
     with zero-padded parity copies of Q (contract over 128, two
     matmuls sharing a stationary); P^T = exp(S^T/8) on ACT; O'^T =
     V_aug^T P^T accumulated in PSUM.  The ACT engine paces this loop,
     so the q-projection of chunk qc+1 and the out-projection of chunk
     qc-1 are chopped into small matmul pieces and interleaved into
     the k-tile stream as PE fillers — PE and ACT both stay ~100%
     busy and no engine idles at chunk boundaries.
"""

import ml_dtypes
import numpy as np

import concourse.bass as bass
import concourse.mybir as mybir
import concourse.tile as tile
from concourse import bacc
from concourse.bass_utils import run_bass_kernel_spmd
from concourse.masks import make_identity

f32 = mybir.dt.float32
bf16 = mybir.dt.bfloat16
np_bf16 = ml_dtypes.bfloat16
AF = mybir.ActivationFunctionType
ALU = mybir.AluOpType

P = 128
MDT = bf16


def build_nc(T=2048, D=1024, HPC=8, HD=64, n_cores=8, debug=False):
    """Build + compile the per-core Bass program. HPC = heads per core."""
    assert HD == 64 and HPC % 2 == 0 and T % 512 == 0 and D % P == 0
    CPC = HPC * HD          # q/out channels per core
    TC = 512                # token chunk (psum free dim), phase A
    QC = 512                # q chunk, attention phase
    NQ = T // TC            # token chunks
    ND = D // P             # model-dim chunks
    NG = HPC // 2           # head pairs
    NKT = T // P            # k-token tiles
    NTT = TC // P           # token tiles per chunk (phase A V)
    HD1 = HD + 1            # V columns + ones column
    SCALE = float(1.0 / np.sqrt(HD))
    ON = min(512, D)        # out-proj free chunk
    NON = D // ON
    NQC = T // QC

    nc = bacc.Bacc("TRN2", target_bir_lowering=False, debug=False,
                   enable_asserts=False, num_devices=n_cores)

    xeT = nc.dram_tensor("x_enc_t", [D, T], MDT, kind="ExternalInput").ap()
    xdT = nc.dram_tensor("x_dec_t", [D, T], MDT, kind="ExternalInput").ap()
    wq_d = nc.dram_tensor("wq", [D, CPC], MDT, kind="ExternalInput").ap()
    wkv_d = nc.dram_tensor("wkv_g", [D, 2 * CPC], MDT, kind="ExternalInput").ap()
    wo_d = nc.dram_tensor("wo", [CPC, D], MDT, kind="ExternalInput").ap()
    bq_d = nc.dram_tensor("bq", [CPC], f32, kind="ExternalInput").ap()
    bkv_d = nc.dram_tensor("bkv_g", [2 * CPC], f32, kind="ExternalInput").ap()
    bo_d = nc.dram_tensor("bo", [D], MDT, kind="ExternalInput").ap()
    # out is stored qt-major: partition p, col (qt*D + c) holds
    # out[qt*128 + p, c] — adjacent qt blocks give 4KB-contiguous
    # per-partition DMA descriptors; the host un-permutes for free
    out_d = nc.dram_tensor("out", [P, (T // P) * D], MDT,
                           kind="ExternalOutput").ap()

    with tile.TileContext(nc) as tc:
      with tc.tile_pool(name="const", bufs=1) as cpool:
        # ---- tiny bias loads first, then wkv, so the first K matmul
        # ---- starts as early as possible
        bo_row = cpool.tile([1, D], MDT, name="bo_row")
        bkv_sb = cpool.tile([P, NG], f32, name="bkv_sb")
        bkv_vrow = cpool.tile([1, CPC], f32, name="bkv_vrow")
        bq_sb = cpool.tile([P, NG], f32, name="bq_sb")
        nc.sync.dma_start(out=bo_row[:], in_=bo_d[:].unsqueeze(0))
        nc.sync.dma_start(out=bkv_vrow[:],
                          in_=bkv_d[CPC:2 * CPC].unsqueeze(0))
        for g in range(NG):
            nc.sync.dma_start(out=bkv_sb[:, g:g + 1],
                              in_=bkv_d[g * P:(g + 1) * P].unsqueeze(1))
            nc.sync.dma_start(out=bq_sb[:, g:g + 1],
                              in_=bq_d[g * P:(g + 1) * P].unsqueeze(1))
        wkv_sb = [cpool.tile([P, 2 * CPC], MDT, name=f"wkv{d}")
                  for d in range(ND)]
        wq_sb = [cpool.tile([P, CPC], MDT, name=f"wq{d}") for d in range(ND)]
        wo_sb = [cpool.tile([P, D], MDT, name=f"wo{g}") for g in range(NG)]

        ident = cpool.tile([P, P], MDT, name="ident")
        make_identity(nc, ident)
        ones_t = cpool.tile([P, P], MDT, name="ones_t")
        nc.vector.tensor_scalar(ones_t[HD:HD + 1, :], ident[HD:HD + 1, :],
                                0.0, 1.0, ALU.mult, ALU.add)
        nc.vector.tensor_scalar(ones_t[0:1, :], ident[0:1, :],
                                0.0, 1.0, ALU.mult, ALU.add)

        # persistent across A->C
        kT = [cpool.tile([P, T], MDT, name=f"kT{g}") for g in range(NG)]
        vnat = [cpool.tile([P, 2 * NKT * HD1], MDT, name=f"vnat{g}")
                for g in range(NG)]     # per pair, head parity h2 in halves
        bo_bc = cpool.tile([P, D], f32, name="bo_bc")
        bkv_vbc = cpool.tile([P, CPC], f32, name="bkv_vbc")

        def vn(h):                      # per-head view [P, NKT*HD1]
            g, h2 = divmod(h, 2)
            off = h2 * NKT * HD1
            return vnat[g][:, off:off + NKT * HD1]

        # ones columns of vnat written once
        for g in range(NG):
            for h2 in range(2):
                blk = vnat[g][:, h2 * NKT * HD1:(h2 + 1) * NKT * HD1] \
                    .rearrange("p (c x) -> p c x", c=NKT)
                nc.vector.tensor_scalar(
                    blk[:, :, HD:HD1], ident[:, 0:NKT].unsqueeze(2),
                    0.0, 1.0, ALU.mult, ALU.add)

        with tc.tile_pool(name="ps_init", bufs=1, space="PSUM") as ips:
            # broadcast bo and the V-part of bkv across partitions via PE
            ps_b = ips.tile([P, D], f32, name="ps_bo")
            for o in range(0, D, 512):
                ow = min(512, D - o)
                nc.tensor.matmul(ps_b[:, o:o + ow], ones_t[0:1, :],
                                 bo_row[0:1, o:o + ow], skip_group_check=True)
            nc.vector.tensor_copy(bo_bc[:], ps_b[:])
            ps_v = ips.tile([P, CPC], f32, name="ps_bkv")
            bkv_vrow16 = cpool.tile([1, CPC], MDT, name="bkv_vrow16")
            nc.vector.tensor_copy(bkv_vrow16[:], bkv_vrow[:])
            nc.tensor.matmul(ps_v[:], ones_t[0:1, :], bkv_vrow16[0:1, :],
                             skip_group_check=True)
            nc.vector.tensor_copy(bkv_vbc[:], ps_v[:])

        # full-row activation tiles (4KB/partition DMA descriptors — the
        # DMA engines are descriptor-rate-bound, so fat rows matter) and
        # padded-parity q tiles live in the const pool
        xeT_sb = [cpool.tile([P, T], MDT, name=f"xeT{d}") for d in range(ND)]
        xdT_sb = [cpool.tile([P, T], MDT, name=f"xdT{d}") for d in range(ND)]
        qTcs = {(qc, g): cpool.tile([P, 2 * QC], MDT, tag=f"qTc{g}", bufs=2,
                                    name=f"qTc_{qc}_{g}")
                for qc in range(NQC) for g in range(NG)}

        def emit_xq_dma(qc, eng=None):
            qcols = slice(qc * QC, (qc + 1) * QC)
            e = eng if eng is not None else nc.sync
            for d in range(ND):
                e.dma_start(out=xdT_sb[d][:, qcols],
                            in_=xdT[d * P:(d + 1) * P, qcols])

        def emit_qproj(qc, g, pool, tag):
            """One filler piece: project q-chunk qc, head pair g."""
            qcols = slice(qc * QC, (qc + 1) * QC)
            pq = pool.tile([P, QC], f32, tag=tag, name=f"pq_{qc}_{g}")
            for d in range(ND):
                nc.tensor.matmul(pq[:], wq_sb[d][:, g * P:(g + 1) * P],
                                 xdT_sb[d][:, qcols],
                                 start=(d == 0), stop=(d == ND - 1),
                                 skip_group_check=True)
            qTc = qTcs[(qc, g)]
            nc.vector.tensor_scalar_add(
                qTc[0:HD, 0:QC], pq[0:HD, :], bq_sb[0:HD, g:g + 1])
            nc.vector.tensor_scalar(
                qTc[HD:P, 0:QC], pq[HD:P, :], 0.0, 0.0, ALU.mult, ALU.add)
            nc.vector.tensor_scalar_add(
                qTc[HD:P, QC:2 * QC], pq[HD:P, :], bq_sb[HD:P, g:g + 1])
            nc.vector.tensor_scalar(
                qTc[0:HD, QC:2 * QC], pq[0:HD, :], 0.0, 0.0, ALU.mult, ALU.add)

        # ---------------- Phase A: x_enc^T -> kT, vnat ----------------
        # d-outer accumulation: all 8 PSUM chains (4 K pairs + 4 V token
        # tiles) advance one d-step as soon as wkv[d] + x[d] land, so the
        # PE starts ~1us after the first DMA instead of waiting for the
        # full weight matrix.
        with tc.tile_pool(name="phA_psK", bufs=1, space="PSUM") as apsK, \
             tc.tile_pool(name="phA_psV", bufs=1, space="PSUM") as apsV:
            for tq in range(NQ - 1):     # last chunk rides inside phase C
                tcols = slice(tq * TC, (tq + 1) * TC)
                # incremental chunked loads: weights on the ACT DGE queue,
                # x_enc chunk-by-chunk on SP, x_dec chunks on ACT behind
                # the weights (all paced one token-chunk ahead of compute)
                for d in range(ND):
                    if tq == 0:
                        nc.scalar.dma_start(out=wkv_sb[d][:],
                                            in_=wkv_d[d * P:(d + 1) * P, :])
                    nc.sync.dma_start(out=xeT_sb[d][:, tcols],
                                      in_=xeT[d * P:(d + 1) * P, tcols])
                if tq == 1:
                    for d in range(ND):
                        nc.scalar.dma_start(out=wq_sb[d][:],
                                            in_=wq_d[d * P:(d + 1) * P, :])
                    emit_xq_dma(0, nc.scalar)
                if tq == 2:
                    for g in range(NG):
                        nc.scalar.dma_start(out=wo_sb[g][:],
                                            in_=wo_d[g * P:(g + 1) * P, :])
                    emit_xq_dma(1, nc.scalar)
                pks = [apsK.tile([P, TC], f32, tag=f"pk{g}",
                                 name=f"pk_{tq}_{g}") for g in range(NG)]
                pvs = [apsV.tile([P, CPC], f32, tag=f"pv{tt}",
                                 name=f"pv_{tq}_{tt}") for tt in range(NTT)]
                for d in range(ND):
                    for g in range(NG):      # K groups: K^T layout
                        nc.tensor.matmul(pks[g][:],
                                         wkv_sb[d][:, g * P:(g + 1) * P],
                                         xeT_sb[d][:, tcols],
                                         start=(d == 0), stop=(d == ND - 1))
                    for tt in range(NTT):    # V natural: x as stationary
                        nc.tensor.matmul(
                            pvs[tt][:],
                            xeT_sb[d][:, tq * TC + tt * P:
                                        tq * TC + (tt + 1) * P],
                            wkv_sb[d][:, CPC:2 * CPC],
                            start=(d == 0), stop=(d == ND - 1))
                for g in range(NG):
                    nc.vector.tensor_scalar_add(
                        kT[g][:, tcols], pks[g][:], bkv_sb[:, g:g + 1])
                for tt in range(NTT):
                    kt_g = tq * NTT + tt     # global k-token tile index
                    for g in range(NG):      # both parities in one DVE op
                        dst = vnat[g][:, :].rearrange(
                            "p (x c y) -> p x c y", x=2, c=NKT)[:, :, kt_g, 0:HD]
                        src = pvs[tt][:, 2 * g * HD:(2 * g + 2) * HD] \
                            .rearrange("p (x y) -> p x y", x=2)
                        bias = bkv_vbc[:, 2 * g * HD:(2 * g + 2) * HD] \
                            .rearrange("p (x y) -> p x y", x=2)
                        nc.vector.tensor_add(dst, src, bias)
            # last k-chunk's x slice: loaded now, consumed by C fillers
            tcols3 = slice((NQ - 1) * TC, NQ * TC)
            for d in range(ND):
                nc.sync.dma_start(out=xeT_sb[d][:, tcols3],
                                  in_=xeT[d * P:(d + 1) * P, tcols3])
            # q-projection for the first attention chunk's g0/g1 rides
            # here; its g2/g3 ride inside chunk 0 itself
            for g in range(2):
                emit_qproj(0, g, apsV, f"pv{g}")

        def tq3_kpiece(g, pool):
            pk = pool.tile([P, TC], f32, tag="pout", name=f"pk3_{g}")
            for d in range(ND):
                nc.tensor.matmul(pk[:], wkv_sb[d][:, g * P:(g + 1) * P],
                                 xeT_sb[d][:, tcols3], start=(d == 0),
                                 stop=(d == ND - 1), skip_group_check=True)
            nc.vector.tensor_scalar_add(kT[g][:, tcols3], pk[:],
                                        bkv_sb[:, g:g + 1])

        def tq3_vpiece(tt, pool):
            kt_g = (NQ - 1) * NTT + tt
            pv = pool.tile([P, CPC], f32, tag="pout", name=f"pv3_{tt}")
            for d in range(ND):
                nc.tensor.matmul(pv[:],
                                 xeT_sb[d][:, (NQ - 1) * TC + tt * P:
                                             (NQ - 1) * TC + (tt + 1) * P],
                                 wkv_sb[d][:, CPC:2 * CPC], start=(d == 0),
                                 stop=(d == ND - 1), skip_group_check=True)
            for g in range(NG):
                dst = vnat[g][:, :].rearrange(
                    "p (x c y) -> p x c y", x=2, c=NKT)[:, :, kt_g, 0:HD]
                src = pv[:, 2 * g * HD:(2 * g + 2) * HD] \
                    .rearrange("p (x y) -> p x y", x=2)
                bias = bkv_vbc[:, 2 * g * HD:(2 * g + 2) * HD] \
                    .rearrange("p (x y) -> p x y", x=2)
                nc.vector.tensor_add(dst, src, bias)

        # ------------- Phase C: attention with interleaved fillers ----------
        with tc.tile_pool(name="phC", bufs=1) as cp2, \
             tc.tile_pool(name="phC_psS", bufs=2, space="PSUM") as psS, \
             tc.tile_pool(name="phC_psO", bufs=2, space="PSUM") as psO, \
             tc.tile_pool(name="phD_ps", bufs=2, space="PSUM") as psD:

            yTss = {}
            ost_hold = {}

            def outproj_piece(qc, qt):
                """Filler piece: one 128-row block of out = Y @ Wo + bo.
                Two qt blocks share one ost tile and leave in one fat DMA."""
                yTs = yTss[qc]
                if qt % 2 == 0:
                    ost_hold[qc] = cp2.tile([P, 2 * D], MDT, tag="ost",
                                            bufs=2, name=f"ost_{qc}_{qt}")
                ost = ost_hold[qc]
                ob = (qt % 2) * D
                for o in range(NON):
                    ocols = slice(o * ON, (o + 1) * ON)
                    pout = psD.tile([P, ON], f32, tag="pout",
                                    name=f"pout_{qc}_{qt}_{o}")
                    for g in range(NG):
                        nc.tensor.matmul(
                            pout[:], yTs[g][:, qt * P:(qt + 1) * P],
                            wo_sb[g][:, ocols],
                            start=(g == 0), stop=(g == NG - 1),
                            skip_group_check=True)
                    nc.vector.tensor_add(ost[:, ob + o * ON:ob + (o + 1) * ON],
                                         pout[:], bo_bc[:, ocols])
                if qt % 2 == 1:
                    qtg = qc * (QC // P) + qt - 1
                    nc.sync.dma_start(
                        out=out_d[:, qtg * D:(qtg + 2) * D], in_=ost[:])

            def qp_piece(qc, g):
                return lambda: emit_qproj(qc, g, psD, "pout")

            def op_piece(qc, qt):
                return lambda: outproj_piece(qc, qt)

            for qc in range(NQC):
                # filler pieces for this chunk's k-tile stream; the last
                # K/V chunk's projections go into chunk 0 as per-head-pair
                # priority pieces (deadline: that pair's kt=12 matmul);
                # out-projections are spread over later chunks so every
                # chunk's filler supply matches its exp-pacing deficit
                gprio = {}
                if qc == 0:
                    # vpieces feed every pair's kt=12..15 -> all inside g0;
                    # kpiece g feeds only pair g's kt=12..15 -> one per g
                    gprio[0] = [lambda tt=tt: tq3_vpiece(tt, psD)
                                for tt in range(NTT)] \
                        + [lambda: tq3_kpiece(0, psD)]
                    gprio[1] = [lambda: tq3_kpiece(1, psD), qp_piece(0, 2)]
                    gprio[2] = [lambda: tq3_kpiece(2, psD), qp_piece(0, 3)]
                    gprio[3] = [lambda: tq3_kpiece(3, psD)]
                    rest = [lambda: emit_xq_dma(2)] \
                        + [qp_piece(1, 0), qp_piece(1, 1)]
                else:
                    # this chunk's own g2/g3 q-projections ride in its g0/g1
                    gprio[0] = [qp_piece(qc, 2)]
                    gprio[1] = [qp_piece(qc, 3)]
                    if qc == 1:
                        rest = [lambda: emit_xq_dma(3)] \
                            + [qp_piece(2, 0), qp_piece(2, 1),
                               op_piece(0, 0), op_piece(0, 1)]
                    elif qc == 2:
                        rest = [qp_piece(3, 0), qp_piece(3, 1),
                                op_piece(0, 2), op_piece(0, 3),
                                op_piece(1, 0)]
                    else:
                        rest = [op_piece(1, 1), op_piece(1, 2),
                                op_piece(1, 3)] \
                            + [op_piece(2, qt) for qt in range(QC // P)]
                # last chunk: front-load so the tail past the final exp
                # holds only the unavoidable last out-projection
                nslots = 24 if qc == NQC - 1 else NG * (NKT // 2)
                nrest = len(rest)
                rpopped = 0
                slot = 0
                cur_prio = []

                def tick():
                    nonlocal slot, rpopped
                    slot += 1
                    if cur_prio:
                        cur_prio.pop(0)()
                        return
                    while rpopped < min(nrest, nrest * slot // nslots):
                        rest[rpopped]()
                        rpopped += 1

                yTs = [cp2.tile([P, QC], MDT, tag=f"yT{g}", bufs=3,
                                name=f"yT_{qc}_{g}") for g in range(NG)]
                yTss[qc] = yTs
                for g in range(NG):
                    cur_prio = gprio.get(g, [])
                    po = [psO.tile([P, QC], f32, tag="po",
                                   name=f"po_{qc}_{g}_{h2}")
                          for h2 in range(2)]
                    pss = {}
                    pts = {}
                    qTc = qTcs[(qc, g)]

                    def mm1(kt, qc=qc, g=g, qTc=qTc, pss=pss):
                        ps = psS.tile([P, 2 * QC], f32, tag="ps",
                                      name=f"ps_{qc}_{g}_{kt}")
                        for h2 in range(2):
                            nc.tensor.matmul(
                                ps[:, h2 * QC:(h2 + 1) * QC],
                                kT[g][:, kt * P:(kt + 1) * P],
                                qTc[:, h2 * QC:(h2 + 1) * QC],
                                skip_group_check=True)
                        pss[kt] = ps

                    def do_exp(kt, qc=qc, g=g, pss=pss, pts=pts):
                        pt = cp2.tile([P, 2 * QC], MDT, tag="pt", bufs=4,
                                      name=f"pt_{qc}_{g}_{kt}")
                        nc.scalar.activation(pt[:], pss[kt][:], AF.Exp,
                                             scale=SCALE)
                        pts[kt] = pt

                    def mm2(kt, g=g, po=po, pts=pts):
                        pt = pts[kt]
                        for h2 in range(2):
                            nc.tensor.matmul(
                                po[h2][0:HD1, :],
                                vn(2 * g + h2)[:, kt * HD1:(kt + 1) * HD1],
                                pt[:, h2 * QC:(h2 + 1) * QC],
                                start=(kt == 0), stop=(kt == NKT - 1),
                                skip_group_check=True)

                    # software pipeline over kt with PE fillers
                    mm1(0)
                    mm1(1)
                    do_exp(0)
                    do_exp(1)
                    for k2 in range(1, NKT // 2):
                        mm1(2 * k2)
                        mm1(2 * k2 + 1)
                        mm2(2 * k2 - 2)
                        mm2(2 * k2 - 1)
                        do_exp(2 * k2)
                        do_exp(2 * k2 + 1)
                        tick()
                    mm2(NKT - 2)
                    mm2(NKT - 1)
                    tick()

                    # normalize + evict to yT
                    srow = [cp2.tile([P, QC], MDT, tag="srow", bufs=2,
                                     name=f"srow_{qc}_{g}_{h2}")
                            for h2 in range(2)]
                    rbc = [cp2.tile([P, QC], f32, tag="rbc", bufs=2,
                                    name=f"rbc_{qc}_{g}_{h2}")
                           for h2 in range(2)]
                    scr = [cp2.tile([P, QC], f32, tag="scr", bufs=2,
                                    name=f"scr_{qc}_{g}_{h2}")
                           for h2 in range(2)]
                    ps_bc = [psD.tile([P, QC], f32, tag="pout",
                                      name=f"psbc_{qc}_{g}_{h2}")
                             for h2 in range(2)]
                    for h2 in range(2):
                        nc.vector.tensor_copy(srow[h2][HD:HD1, :],
                                              po[h2][HD:HD1, :])
                    for h2 in range(2):
                        nc.tensor.matmul(ps_bc[h2][0:HD, :],
                                         ones_t[HD:HD1, 0:HD],
                                         srow[h2][HD:HD1, :],
                                         skip_group_check=True)
                    for h2 in range(2):
                        nc.vector.reciprocal_approx_accurate(
                            out=rbc[h2][0:HD, :], in_=ps_bc[h2][0:HD, :],
                            scratch=scr[h2][0:HD, :])
                    nc.vector.tensor_mul(yTs[g][0:HD, :],
                                         po[0][0:HD, :], rbc[0][0:HD, :])
                    stg = cp2.tile([P, QC], MDT, tag="stg",
                                   bufs=2, name=f"stg_{qc}_{g}")
                    nc.vector.tensor_mul(stg[0:HD, :],
                                         po[1][0:HD, :], rbc[1][0:HD, :])
                    nc.sync.dma_start(out=yTs[g][HD:P, :],
                                      in_=stg[0:HD, :])
                # leftovers (shouldn't happen, but keep it correct)
                for g in range(NG):
                    while gprio.get(g):
                        gprio[g].pop(0)()
                while rpopped < nrest:
                    rest[rpopped]()
                    rpopped += 1

            # tail: out-projection of the last chunk
            for qt in range(QC // P):
                outproj_piece(NQC - 1, qt)

    nc.compile()
    return nc


# ---------------------------------------------------------------------------
# Host side: sharding, run, unshard
# ---------------------------------------------------------------------------

_NC_CACHE = {}


def _get_nc():
    key = "full"
    if key not in _NC_CACHE:
        _NC_CACHE[key] = build_nc()
    return _NC_CACHE[key]


def _group_kv_cols(w_slice, HPC, HD):
    """Reorder kv columns [h-major, (k|v), d] -> K head-pair groups then V."""
    last = w_slice.shape[-1]
    assert last == HPC * 2 * HD
    arr = w_slice.reshape(w_slice.shape[:-1] + (HPC, 2, HD))
    kpart = arr[..., :, 0, :].reshape(w_slice.shape[:-1] + (HPC * HD,))
    vpart = arr[..., :, 1, :].reshape(w_slice.shape[:-1] + (HPC * HD,))
    return np.ascontiguousarray(np.concatenate([kpart, vpart], axis=-1))


def make_in_maps(x_enc, x_dec, Wq, bq, Wkv, bkv, Wo, bo, n_cores=8,
                 HPC=8, HD=64):
    CPC = HPC * HD
    mdt = np_bf16
    in_maps = []
    xet = [np.ascontiguousarray(x_enc[b].T).astype(mdt)
           for b in range(x_enc.shape[0])]
    xdt = [np.ascontiguousarray(x_dec[b].T).astype(mdt)
           for b in range(x_dec.shape[0])]
    for c in range(n_cores):
        b, hg = c // 2, c % 2
        wkv_slice = Wkv[:, hg * 2 * CPC:(hg + 1) * 2 * CPC]
        bkv_slice = bkv[hg * 2 * CPC:(hg + 1) * 2 * CPC]
        in_maps.append({
            "x_enc_t": xet[b],
            "x_dec_t": xdt[b],
            "wq": np.ascontiguousarray(Wq[:, hg * CPC:(hg + 1) * CPC]).astype(mdt),
            "wkv_g": _group_kv_cols(wkv_slice, HPC, HD).astype(mdt),
            "wo": np.ascontiguousarray(Wo[hg * CPC:(hg + 1) * CPC, :]).astype(mdt),
            "bq": np.ascontiguousarray(bq[hg * CPC:(hg + 1) * CPC]),
            "bkv_g": _group_kv_cols(bkv_slice, HPC, HD),
            "bo": (np.ascontiguousarray(bo) if hg == 0
                   else np.zeros_like(bo)).astype(mdt),
        })
    return in_maps


def unshard(outs, B=4, T=2048, D=1024):
    def unpack(a):
        # device layout: [128, (T//128)*D] qt-major -> [T, D]
        return np.ascontiguousarray(
            a.astype(np.float32).reshape(P, T // P, D).transpose(1, 0, 2)
            .reshape(T, D))
    return np.stack([unpack(outs[2 * b]) + unpack(outs[2 * b + 1])
                     for b in range(B)], axis=0)


def kernel(x_enc, x_dec, mask, Wq, bq, Wkv, bkv, Wo, bo):
    x_enc = np.asarray(x_enc, dtype=np.float32)
    x_dec = np.asarray(x_dec, dtype=np.float32)
    Wq = np.asarray(Wq, dtype=np.float32)
    bq = np.asarray(bq, dtype=np.float32)
    Wkv = np.asarray(Wkv, dtype=np.float32)
    bkv = np.asarray(bkv, dtype=np.float32)
    Wo = np.asarray(Wo, dtype=np.float32)
    bo = np.asarray(bo, dtype=np.float32)
    mask = np.asarray(mask)
    if mask.any():
        raise ValueError("kernel assumes a zero additive mask (spec fill=zeros)")

    nc = _get_nc()
    in_maps = make_in_maps(x_enc, x_dec, Wq, bq, Wkv, bkv, Wo, bo)
    res = run_bass_kernel_spmd(nc, in_maps, core_ids=list(range(8)))
    outs = [res.results[c]["out"] for c in range(8)]
    return unshard(outs, B=x_enc.shape[0])


if __name__ == "__main__":
    import time
    t0 = time.time()
    nc = _get_nc()
    print(f"build+compile ok in {time.time() - t0:.1f}s")


# revision 58
# speedup vs baseline: 1.0342x; 1.0008x over previous
"""Multi-head cross-attention on Trainium2, 8-core SPMD.

Problem (hardcoded): B=4, T=2048, D=1024, H=16 heads, head_dim=64, fp32.
    kv = x_enc @ Wkv + bkv ; q = x_dec @ Wq + bq
    per head: S = q_h k_h^T / sqrt(64); P = softmax(S + mask); O_h = P v_h
    out = concat_h(O_h) @ Wo + bo

Sharding: data parallel over batch (4 slices x 2 cores each) and tensor
parallel over heads within each pair (8 heads per core).  Each core
computes a partial output  Y_local @ Wo[rows_local]  (+bo on the even
core of the pair); the host unshards by summing each pair's partials
and stacking the 4 batch slices.  Host-side shard prep pre-transposes
the activations, regroups Wkv columns, and casts matmul operands to
bf16 (PSUM accumulation stays fp32 on device).

The mask input is structurally zero for this problem (spec fill
"zeros"); softmax(S + 0) == softmax(S), so the kernel does not load it
(checked on the host).

Per-core schedule (bf16 operands, fp32 accumulation):
  A: K^T = Wkv_K^T x_enc^T (head-pair-stacked on partitions); V in
     natural layout directly via x-stationary matmuls, next to a ones
     column that later yields the softmax denominator for free.
     Q-projection for the first q-chunk rides at the end of A.
  C: per q-chunk qc, per head pair, per k-tile: S^T = (K^T tile)^T Q^T
     with zero-padded parity copies of Q (contract over 128, two
     matmuls sharing a stationary); P^T = exp(S^T/8) on ACT; O'^T =
     V_aug^T P^T accumulated in PSUM.  The ACT engine paces this loop,
     so the q-projection of chunk qc+1 and the out-projection of chunk
     qc-1 are chopped into small matmul pieces and interleaved into
     the k-tile stream as PE fillers — PE and ACT both stay ~100%
     busy and no engine idles at chunk boundaries.
"""

import ml_dtypes
import numpy as np

import concourse.bass as bass
import concourse.mybir as mybir
import concourse.tile as tile
from concourse import bacc
from concourse.bass_utils import run_bass_kernel_spmd
from concourse.masks import make_identity

f32 = mybir.dt.float32
bf16 = mybir.dt.bfloat16
np_bf16 = ml_dtypes.bfloat16
AF = mybir.ActivationFunctionType
ALU = mybir.AluOpType

P = 128
MDT = bf16


def build_nc(T=2048, D=1024, HPC=8, HD=64, n_cores=8, debug=False):
    """Build + compile the per-core Bass program. HPC = heads per core."""
    assert HD == 64 and HPC % 2 == 0 and T % 512 == 0 and D % P == 0
    CPC = HPC * HD          # q/out channels per core
    TC = 512                # token chunk (psum free dim), phase A
    QC = 512                # q chunk, attention phase
    NQ = T // TC            # token chunks
    ND = D // P             # model-dim chunks
    NG = HPC // 2           # head pairs
    NKT = T // P            # k-token tiles
    NTT = TC // P           # token tiles per chunk (phase A V)
    HD1 = HD + 1            # V columns + ones column
    SCALE = float(1.0 / np.sqrt(HD))
    ON = min(512, D)        # out-proj free chunk
    NON = D // ON
    NQC = T // QC

    nc = bacc.Bacc("TRN2", target_bir_lowering=False, debug=False,
                   enable_asserts=False, num_devices=n_cores)

    xeT = nc.dram_tensor("x_enc_t", [D, T], MDT, kind="ExternalInput").ap()
    xdT = nc.dram_tensor("x_dec_t", [D, T], MDT, kind="ExternalInput").ap()
    wq_d = nc.dram_tensor("wq", [D, CPC], MDT, kind="ExternalInput").ap()
    wkv_d = nc.dram_tensor("wkv_g", [D, 2 * CPC], MDT, kind="ExternalInput").ap()
    wo_d = nc.dram_tensor("wo", [CPC, D], MDT, kind="ExternalInput").ap()
    bq_d = nc.dram_tensor("bq", [CPC], f32, kind="ExternalInput").ap()
    bkv_d = nc.dram_tensor("bkv_g", [2 * CPC], f32, kind="ExternalInput").ap()
    bo_d = nc.dram_tensor("bo", [D], MDT, kind="ExternalInput").ap()
    # out is stored qt-major: partition p, col (qt*D + c) holds
    # out[qt*128 + p, c] — adjacent qt blocks give 4KB-contiguous
    # per-partition DMA descriptors; the host un-permutes for free
    out_d = nc.dram_tensor("out", [P, (T // P) * D], MDT,
                           kind="ExternalOutput").ap()

    with tile.TileContext(nc) as tc:
      with tc.tile_pool(name="const", bufs=1) as cpool:
        # ---- tiny bias loads first, then wkv, so the first K matmul
        # ---- starts as early as possible
        bo_row = cpool.tile([1, D], MDT, name="bo_row")
        bkv_sb = cpool.tile([P, NG], f32, name="bkv_sb")
        bkv_vrow = cpool.tile([1, CPC], f32, name="bkv_vrow")
        bq_sb = cpool.tile([P, NG], f32, name="bq_sb")
        nc.sync.dma_start(out=bo_row[:], in_=bo_d[:].unsqueeze(0))
        nc.sync.dma_start(out=bkv_vrow[:],
                          in_=bkv_d[CPC:2 * CPC].unsqueeze(0))
        for g in range(NG):
            nc.sync.dma_start(out=bkv_sb[:, g:g + 1],
                              in_=bkv_d[g * P:(g + 1) * P].unsqueeze(1))
            nc.sync.dma_start(out=bq_sb[:, g:g + 1],
                              in_=bq_d[g * P:(g + 1) * P].unsqueeze(1))
        wkv_sb = [cpool.tile([P, 2 * CPC], MDT, name=f"wkv{d}")
                  for d in range(ND)]
        wq_sb = [cpool.tile([P, CPC], MDT, name=f"wq{d}") for d in range(ND)]
        wo_sb = [cpool.tile([P, D], MDT, name=f"wo{g}") for g in range(NG)]

        ident = cpool.tile([P, P], MDT, name="ident")
        make_identity(nc, ident)
        ones_t = cpool.tile([P, P], MDT, name="ones_t")
        nc.vector.tensor_scalar(ones_t[HD:HD + 1, :], ident[HD:HD + 1, :],
                                0.0, 1.0, ALU.mult, ALU.add)
        nc.vector.tensor_scalar(ones_t[0:1, :], ident[0:1, :],
                                0.0, 1.0, ALU.mult, ALU.add)

        # persistent across A->C
        kT = [cpool.tile([P, T], MDT, name=f"kT{g}") for g in range(NG)]
        vnat = [cpool.tile([P, 2 * NKT * HD1], MDT, name=f"vnat{g}")
                for g in range(NG)]     # per pair, head parity h2 in halves
        bo_bc = cpool.tile([P, D], f32, name="bo_bc")
        bkv_vbc = cpool.tile([P, CPC], f32, name="bkv_vbc")

        def vn(h):                      # per-head view [P, NKT*HD1]
            g, h2 = divmod(h, 2)
            off = h2 * NKT * HD1
            return vnat[g][:, off:off + NKT * HD1]

        # ones columns of vnat written once
        for g in range(NG):
            for h2 in range(2):
                blk = vnat[g][:, h2 * NKT * HD1:(h2 + 1) * NKT * HD1] \
                    .rearrange("p (c x) -> p c x", c=NKT)
                nc.vector.tensor_scalar(
                    blk[:, :, HD:HD1], ident[:, 0:NKT].unsqueeze(2),
                    0.0, 1.0, ALU.mult, ALU.add)

        with tc.tile_pool(name="ps_init", bufs=1, space="PSUM") as ips:
            # broadcast bo and the V-part of bkv across partitions via PE
            ps_b = ips.tile([P, D], f32, name="ps_bo")
            for o in range(0, D, 512):
                ow = min(512, D - o)
                nc.tensor.matmul(ps_b[:, o:o + ow], ones_t[0:1, :],
                                 bo_row[0:1, o:o + ow], skip_group_check=True)
            nc.vector.tensor_copy(bo_bc[:], ps_b[:])
            ps_v = ips.tile([P, CPC], f32, name="ps_bkv")
            bkv_vrow16 = cpool.tile([1, CPC], MDT, name="bkv_vrow16")
            nc.vector.tensor_copy(bkv_vrow16[:], bkv_vrow[:])
            nc.tensor.matmul(ps_v[:], ones_t[0:1, :], bkv_vrow16[0:1, :],
                             skip_group_check=True)
            nc.vector.tensor_copy(bkv_vbc[:], ps_v[:])

        # full-row activation tiles (4KB/partition DMA descriptors — the
        # DMA engines are descriptor-rate-bound, so fat rows matter) and
        # padded-parity q tiles live in the const pool
        xeT_sb = [cpool.tile([P, T], MDT, name=f"xeT{d}") for d in range(ND)]
        xdT_sb = [cpool.tile([P, T], MDT, name=f"xdT{d}") for d in range(ND)]
        qTcs = {(qc, g): cpool.tile([P, 2 * QC], MDT, tag=f"qTc{g}", bufs=2,
                                    name=f"qTc_{qc}_{g}")
                for qc in range(NQC) for g in range(NG)}

        def emit_xq_dma(qc, eng=None):
            qcols = slice(qc * QC, (qc + 1) * QC)
            e = eng if eng is not None else nc.sync
            for d in range(ND):
                e.dma_start(out=xdT_sb[d][:, qcols],
                            in_=xdT[d * P:(d + 1) * P, qcols])

        def emit_qproj(qc, g, pool, tag):
            """One filler piece: project q-chunk qc, head pair g."""
            qcols = slice(qc * QC, (qc + 1) * QC)
            pq = pool.tile([P, QC], f32, tag=tag, name=f"pq_{qc}_{g}")
            for d in range(ND):
                nc.tensor.matmul(pq[:], wq_sb[d][:, g * P:(g + 1) * P],
                                 xdT_sb[d][:, qcols],
                                 start=(d == 0), stop=(d == ND - 1),
                                 skip_group_check=True)
            qTc = qTcs[(qc, g)]
            nc.vector.tensor_scalar_add(
                qTc[0:HD, 0:QC], pq[0:HD, :], bq_sb[0:HD, g:g + 1])
            nc.vector.tensor_scalar(
                qTc[HD:P, 0:QC], pq[HD:P, :], 0.0, 0.0, ALU.mult, ALU.add)
            nc.vector.tensor_scalar_add(
                qTc[HD:P, QC:2 * QC], pq[HD:P, :], bq_sb[HD:P, g:g + 1])
            nc.vector.tensor_scalar(
                qTc[0:HD, QC:2 * QC], pq[0:HD, :], 0.0, 0.0, ALU.mult, ALU.add)

        # ---------------- Phase A: x_enc^T -> kT, vnat ----------------
        # d-outer accumulation: all 8 PSUM chains (4 K pairs + 4 V token
        # tiles) advance one d-step as soon as wkv[d] + x[d] land, so the
        # PE starts ~1us after the first DMA instead of waiting for the
        # full weight matrix.
        with tc.tile_pool(name="phA_psK", bufs=1, space="PSUM") as apsK, \
             tc.tile_pool(name="phA_psV", bufs=1, space="PSUM") as apsV:
            for tq in range(NQ - 1):     # last chunk rides inside phase C
                tcols = slice(tq * TC, (tq + 1) * TC)
                # incremental chunked loads: weights on the ACT DGE queue,
                # x_enc chunk-by-chunk on SP, x_dec chunks on ACT behind
                # the weights (all paced one token-chunk ahead of compute).
                # tq0 loads the K-half of wkv first so the K chains can
                # finish after ~2MB of DMA instead of 3MB.
                for d in range(ND):
                    if tq == 0:
                        nc.scalar.dma_start(
                            out=wkv_sb[d][:, 0:CPC],
                            in_=wkv_d[d * P:(d + 1) * P, 0:CPC])
                    nc.sync.dma_start(out=xeT_sb[d][:, tcols],
                                      in_=xeT[d * P:(d + 1) * P, tcols])
                if tq == 0:
                    for d in range(ND):
                        nc.scalar.dma_start(
                            out=wkv_sb[d][:, CPC:2 * CPC],
                            in_=wkv_d[d * P:(d + 1) * P, CPC:2 * CPC])
                if tq == 1:
                    for d in range(ND):
                        nc.scalar.dma_start(out=wq_sb[d][:],
                                            in_=wq_d[d * P:(d + 1) * P, :])
                    emit_xq_dma(0, nc.scalar)
                if tq == 2:
                    for g in range(NG):
                        nc.scalar.dma_start(out=wo_sb[g][:],
                                            in_=wo_d[g * P:(g + 1) * P, :])
                    emit_xq_dma(1, nc.scalar)
                pks = [apsK.tile([P, TC], f32, tag=f"pk{g}",
                                 name=f"pk_{tq}_{g}") for g in range(NG)]
                pvs = [apsV.tile([P, CPC], f32, tag=f"pv{tt}",
                                 name=f"pv_{tq}_{tt}") for tt in range(NTT)]
                # tq0 runs K chains before V chains (K weights land first)
                passes = [(True, False), (False, True)] if tq == 0 \
                    else [(True, True)]
                for do_k, do_v in passes:
                    for d in range(ND):
                        if do_k:
                            for g in range(NG):      # K groups: K^T layout
                                nc.tensor.matmul(
                                    pks[g][:],
                                    wkv_sb[d][:, g * P:(g + 1) * P],
                                    xeT_sb[d][:, tcols],
                                    start=(d == 0), stop=(d == ND - 1))
                        if do_v:
                            for tt in range(NTT):    # V natural
                                nc.tensor.matmul(
                                    pvs[tt][:],
                                    xeT_sb[d][:, tq * TC + tt * P:
                                                tq * TC + (tt + 1) * P],
                                    wkv_sb[d][:, CPC:2 * CPC],
                                    start=(d == 0), stop=(d == ND - 1))
                for g in range(NG):
                    nc.vector.tensor_scalar_add(
                        kT[g][:, tcols], pks[g][:], bkv_sb[:, g:g + 1])
                for tt in range(NTT):
                    kt_g = tq * NTT + tt     # global k-token tile index
                    for g in range(NG):      # both parities in one DVE op
                        dst = vnat[g][:, :].rearrange(
                            "p (x c y) -> p x c y", x=2, c=NKT)[:, :, kt_g, 0:HD]
                        src = pvs[tt][:, 2 * g * HD:(2 * g + 2) * HD] \
                            .rearrange("p (x y) -> p x y", x=2)
                        bias = bkv_vbc[:, 2 * g * HD:(2 * g + 2) * HD] \
                            .rearrange("p (x y) -> p x y", x=2)
                        nc.vector.tensor_add(dst, src, bias)
            # last k-chunk's x slice: loaded now, consumed by C fillers
            tcols3 = slice((NQ - 1) * TC, NQ * TC)
            for d in range(ND):
                nc.sync.dma_start(out=xeT_sb[d][:, tcols3],
                                  in_=xeT[d * P:(d + 1) * P, tcols3])
            # q-projection for the first attention chunk's g0/g1 rides
            # here; its g2/g3 ride inside chunk 0 itself
            for g in range(2):
                emit_qproj(0, g, apsV, f"pv{g}")

        def tq3_kpiece(g, pool):
            pk = pool.tile([P, TC], f32, tag="pout", name=f"pk3_{g}")
            for d in range(ND):
                nc.tensor.matmul(pk[:], wkv_sb[d][:, g * P:(g + 1) * P],
                                 xeT_sb[d][:, tcols3], start=(d == 0),
                                 stop=(d == ND - 1), skip_group_check=True)
            nc.vector.tensor_scalar_add(kT[g][:, tcols3], pk[:],
                                        bkv_sb[:, g:g + 1])

        def tq3_vpiece(tt, pool):
            kt_g = (NQ - 1) * NTT + tt
            pv = pool.tile([P, CPC], f32, tag="pout", name=f"pv3_{tt}")
            for d in range(ND):
                nc.tensor.matmul(pv[:],
                                 xeT_sb[d][:, (NQ - 1) * TC + tt * P:
                                             (NQ - 1) * TC + (tt + 1) * P],
                                 wkv_sb[d][:, CPC:2 * CPC], start=(d == 0),
                                 stop=(d == ND - 1), skip_group_check=True)
            for g in range(NG):
                dst = vnat[g][:, :].rearrange(
                    "p (x c y) -> p x c y", x=2, c=NKT)[:, :, kt_g, 0:HD]
                src = pv[:, 2 * g * HD:(2 * g + 2) * HD] \
                    .rearrange("p (x y) -> p x y", x=2)
                bias = bkv_vbc[:, 2 * g * HD:(2 * g + 2) * HD] \
                    .rearrange("p (x y) -> p x y", x=2)
                nc.vector.tensor_add(dst, src, bias)

        # ------------- Phase C: attention with interleaved fillers ----------
        with tc.tile_pool(name="phC", bufs=1) as cp2, \
             tc.tile_pool(name="phC_psS", bufs=2, space="PSUM") as psS, \
             tc.tile_pool(name="phC_psO", bufs=2, space="PSUM") as psO, \
             tc.tile_pool(name="phD_ps", bufs=2, space="PSUM") as psD:

            yTss = {}
            ost_hold = {}

            def outproj_piece(qc, qt):
                """Filler piece: one 128-row block of out = Y @ Wo + bo.
                Two qt blocks share one ost tile and leave in one fat DMA."""
                yTs = yTss[qc]
                if qt % 2 == 0:
                    ost_hold[qc] = cp2.tile([P, 2 * D], MDT, tag="ost",
                                            bufs=2, name=f"ost_{qc}_{qt}")
                ost = ost_hold[qc]
                ob = (qt % 2) * D
                for o in range(NON):
                    ocols = slice(o * ON, (o + 1) * ON)
                    pout = psD.tile([P, ON], f32, tag="pout",
                                    name=f"pout_{qc}_{qt}_{o}")
                    for g in range(NG):
                        nc.tensor.matmul(
                            pout[:], yTs[g][:, qt * P:(qt + 1) * P],
                            wo_sb[g][:, ocols],
                            start=(g == 0), stop=(g == NG - 1),
                            skip_group_check=True)
                    nc.vector.tensor_add(ost[:, ob + o * ON:ob + (o + 1) * ON],
                                         pout[:], bo_bc[:, ocols])
                if qt % 2 == 1:
                    qtg = qc * (QC // P) + qt - 1
                    # the final chunk's output drains on the by-then-idle
                    # ACT DGE queue, in parallel with the SP queue
                    eng = nc.scalar if qc == NQC - 1 and qt == 3 else nc.sync
                    eng.dma_start(
                        out=out_d[:, qtg * D:(qtg + 2) * D], in_=ost[:])

            def qp_piece(qc, g):
                return lambda: emit_qproj(qc, g, psD, "pout")

            def op_piece(qc, qt):
                return lambda: outproj_piece(qc, qt)

            for qc in range(NQC):
                # filler pieces for this chunk's k-tile stream; the last
                # K/V chunk's projections go into chunk 0 as per-head-pair
                # priority pieces (deadline: that pair's kt=12 matmul);
                # out-projections are spread over later chunks so every
                # chunk's filler supply matches its exp-pacing deficit
                gprio = {}
                if qc == 0:
                    # vpieces feed every pair's kt=12..15 -> all inside g0;
                    # kpiece g feeds only pair g's kt=12..15 -> one per g
                    gprio[0] = [lambda tt=tt: tq3_vpiece(tt, psD)
                                for tt in range(NTT)] \
                        + [lambda: tq3_kpiece(0, psD)]
                    gprio[1] = [lambda: tq3_kpiece(1, psD), qp_piece(0, 2)]
                    gprio[2] = [lambda: tq3_kpiece(2, psD), qp_piece(0, 3)]
                    gprio[3] = [lambda: tq3_kpiece(3, psD)]
                    rest = [lambda: emit_xq_dma(2)] \
                        + [qp_piece(1, 0), qp_piece(1, 1)]
                else:
                    # this chunk's own g2/g3 q-projections ride in its g0/g1
                    gprio[0] = [qp_piece(qc, 2)]
                    gprio[1] = [qp_piece(qc, 3)]
                    if qc == 1:
                        rest = [lambda: emit_xq_dma(3)] \
                            + [qp_piece(2, 0), qp_piece(2, 1),
                               op_piece(0, 0), op_piece(0, 1)]
                    elif qc == 2:
                        rest = [qp_piece(3, 0), qp_piece(3, 1),
                                op_piece(0, 2), op_piece(0, 3),
                                op_piece(1, 0)]
                    else:
                        rest = [op_piece(1, 1), op_piece(1, 2),
                                op_piece(1, 3)] \
                            + [op_piece(2, qt) for qt in range(QC // P)]
                nslots = NG * (NKT // 2)
                nrest = len(rest)
                rpopped = 0
                slot = 0
                cur_prio = []

                def tick():
                    nonlocal slot, rpopped
                    slot += 1
                    if cur_prio:
                        cur_prio.pop(0)()
                        return
                    while rpopped < min(nrest, nrest * slot // nslots):
                        rest[rpopped]()
                        rpopped += 1

                yTs = [cp2.tile([P, QC], MDT, tag=f"yT{g}", bufs=3,
                                name=f"yT_{qc}_{g}") for g in range(NG)]
                yTss[qc] = yTs
                for g in range(NG):
                    cur_prio = gprio.get(g, [])
                    po = [psO.tile([P, QC], f32, tag="po",
                                   name=f"po_{qc}_{g}_{h2}")
                          for h2 in range(2)]
                    pss = {}
                    pts = {}
                    qTc = qTcs[(qc, g)]

                    def mm1(kt, qc=qc, g=g, qTc=qTc, pss=pss):
                        ps = psS.tile([P, 2 * QC], f32, tag="ps",
                                      name=f"ps_{qc}_{g}_{kt}")
                        for h2 in range(2):
                            nc.tensor.matmul(
                                ps[:, h2 * QC:(h2 + 1) * QC],
                                kT[g][:, kt * P:(kt + 1) * P],
                                qTc[:, h2 * QC:(h2 + 1) * QC],
                                skip_group_check=True)
                        pss[kt] = ps

                    def do_exp(kt, qc=qc, g=g, pss=pss, pts=pts):
                        pt = cp2.tile([P, 2 * QC], MDT, tag="pt", bufs=4,
                                      name=f"pt_{qc}_{g}_{kt}")
                        nc.scalar.activation(pt[:], pss[kt][:], AF.Exp,
                                             scale=SCALE)
                        pts[kt] = pt

                    def mm2(kt, g=g, po=po, pts=pts):
                        pt = pts[kt]
                        for h2 in range(2):
                            nc.tensor.matmul(
                                po[h2][0:HD1, :],
                                vn(2 * g + h2)[:, kt * HD1:(kt + 1) * HD1],
                                pt[:, h2 * QC:(h2 + 1) * QC],
                                start=(kt == 0), stop=(kt == NKT - 1),
                                skip_group_check=True)

                    # software pipeline over kt with PE fillers
                    mm1(0)
                    mm1(1)
                    do_exp(0)
                    do_exp(1)
                    for k2 in range(1, NKT // 2):
                        mm1(2 * k2)
                        mm1(2 * k2 + 1)
                        mm2(2 * k2 - 2)
                        mm2(2 * k2 - 1)
                        do_exp(2 * k2)
                        do_exp(2 * k2 + 1)
                        tick()
                    mm2(NKT - 2)
                    mm2(NKT - 1)
                    tick()

                    # normalize + evict to yT
                    srow = [cp2.tile([P, QC], MDT, tag="srow", bufs=2,
                                     name=f"srow_{qc}_{g}_{h2}")
                            for h2 in range(2)]
                    rbc = [cp2.tile([P, QC], f32, tag="rbc", bufs=2,
                                    name=f"rbc_{qc}_{g}_{h2}")
                           for h2 in range(2)]
                    scr = [cp2.tile([P, QC], f32, tag="scr", bufs=2,
                                    name=f"scr_{qc}_{g}_{h2}")
                           for h2 in range(2)]
                    ps_bc = [psD.tile([P, QC], f32, tag="pout",
                                      name=f"psbc_{qc}_{g}_{h2}")
                             for h2 in range(2)]
                    for h2 in range(2):
                        nc.vector.tensor_copy(srow[h2][HD:HD1, :],
                                              po[h2][HD:HD1, :])
                    for h2 in range(2):
                        nc.tensor.matmul(ps_bc[h2][0:HD, :],
                                         ones_t[HD:HD1, 0:HD],
                                         srow[h2][HD:HD1, :],
                                         skip_group_check=True)
                    for h2 in range(2):
                        nc.vector.reciprocal_approx_accurate(
                            out=rbc[h2][0:HD, :], in_=ps_bc[h2][0:HD, :],
                            scratch=scr[h2][0:HD, :])
                    nc.vector.tensor_mul(yTs[g][0:HD, :],
                                         po[0][0:HD, :], rbc[0][0:HD, :])
                    stg = cp2.tile([P, QC], MDT, tag="stg",
                                   bufs=2, name=f"stg_{qc}_{g}")
                    nc.vector.tensor_mul(stg[0:HD, :],
                                         po[1][0:HD, :], rbc[1][0:HD, :])
                    nc.sync.dma_start(out=yTs[g][HD:P, :],
                                      in_=stg[0:HD, :])
                # leftovers (shouldn't happen, but keep it correct)
                for g in range(NG):
                    while gprio.get(g):
                        gprio[g].pop(0)()
                while rpopped < nrest:
                    rest[rpopped]()
                    rpopped += 1

            # tail: out-projection of the last chunk
            for qt in range(QC // P):
                outproj_piece(NQC - 1, qt)

    nc.compile()
    return nc


# ---------------------------------------------------------------------------
# Host side: sharding, run, unshard
# ---------------------------------------------------------------------------

_NC_CACHE = {}


def _get_nc():
    key = "full"
    if key not in _NC_CACHE:
        _NC_CACHE[key] = build_nc()
    return _NC_CACHE[key]


def _group_kv_cols(w_slice, HPC, HD):
    """Reorder kv columns [h-major, (k|v), d] -> K head-pair groups then V."""
    last = w_slice.shape[-1]
    assert last == HPC * 2 * HD
    arr = w_slice.reshape(w_slice.shape[:-1] + (HPC, 2, HD))
    kpart = arr[..., :, 0, :].reshape(w_slice.shape[:-1] + (HPC * HD,))
    vpart = arr[..., :, 1, :].reshape(w_slice.shape[:-1] + (HPC * HD,))
    return np.ascontiguousarray(np.concatenate([kpart, vpart], axis=-1))


def make_in_maps(x_enc, x_dec, Wq, bq, Wkv, bkv, Wo, bo, n_cores=8,
                 HPC=8, HD=64):
    CPC = HPC * HD
    mdt = np_bf16
    in_maps = []
    xet = [np.ascontiguousarray(x_enc[b].T).astype(mdt)
           for b in range(x_enc.shape[0])]
    xdt = [np.ascontiguousarray(x_dec[b].T).astype(mdt)
           for b in range(x_dec.shape[0])]
    for c in range(n_cores):
        b, hg = c // 2, c % 2
        wkv_slice = Wkv[:, hg * 2 * CPC:(hg + 1) * 2 * CPC]
        bkv_slice = bkv[hg * 2 * CPC:(hg + 1) * 2 * CPC]
        in_maps.append({
            "x_enc_t": xet[b],
            "x_dec_t": xdt[b],
            "wq": np.ascontiguousarray(Wq[:, hg * CPC:(hg + 1) * CPC]).astype(mdt),
            "wkv_g": _group_kv_cols(wkv_slice, HPC, HD).astype(mdt),
            "wo": np.ascontiguousarray(Wo[hg * CPC:(hg + 1) * CPC, :]).astype(mdt),
            "bq": np.ascontiguousarray(bq[hg * CPC:(hg + 1) * CPC]),
            "bkv_g": _group_kv_cols(bkv_slice, HPC, HD),
            "bo": (np.ascontiguousarray(bo) if hg == 0
                   else np.zeros_like(bo)).astype(mdt),
        })
    return in_maps


def unshard(outs, B=4, T=2048, D=1024):
    def unpack(a):
        # device layout: [128, (T//128)*D] qt-major -> [T, D]
        return np.ascontiguousarray(
            a.astype(np.float32).reshape(P, T // P, D).transpose(1, 0, 2)
            .reshape(T, D))
    return np.stack([unpack(outs[2 * b]) + unpack(outs[2 * b + 1])
                     for b in range(B)], axis=0)


def kernel(x_enc, x_dec, mask, Wq, bq, Wkv, bkv, Wo, bo):
    x_enc = np.asarray(x_enc, dtype=np.float32)
    x_dec = np.asarray(x_dec, dtype=np.float32)
    Wq = np.asarray(Wq, dtype=np.float32)
    bq = np.asarray(bq, dtype=np.float32)
    Wkv = np.asarray(Wkv, dtype=np.float32)
    bkv = np.asarray(bkv, dtype=np.float32)
    Wo = np.asarray(Wo, dtype=np.float32)
    bo = np.asarray(bo, dtype=np.float32)
    mask = np.asarray(mask)
    if mask.any():
        raise ValueError("kernel assumes a zero additive mask (spec fill=zeros)")

    nc = _get_nc()
    in_maps = make_in_maps(x_enc, x_dec, Wq, bq, Wkv, bkv, Wo, bo)
    res = run_bass_kernel_spmd(nc, in_maps, core_ids=list(range(8)))
    outs = [res.results[c]["out"] for c in range(8)]
    return unshard(outs, B=x_enc.shape[0])


if __name__ == "__main__":
    import time
    t0 = time.time()
    nc = _get_nc()
    print(f"build+compile ok in {time.time() - t0:.1f}s")


# revision 63
# speedup vs baseline: 1.0388x; 1.0044x over previous
"""Multi-head cross-attention on Trainium2, 8-core SPMD.

Problem (hardcoded): B=4, T=2048, D=1024, H=16 heads, head_dim=64, fp32.
    kv = x_enc @ Wkv + bkv ; q = x_dec @ Wq + bq
    per head: S = q_h k_h^T / sqrt(64); P = softmax(S + mask); O_h = P v_h
    out = concat_h(O_h) @ Wo + bo

Sharding: data parallel over batch (4 slices x 2 cores each) and tensor
parallel over heads within each pair (8 heads per core).  Each core
computes a partial output  Y_local @ Wo[rows_local]  (+bo on the even
core of the pair); the host unshards by summing each pair's partials
and stacking the 4 batch slices.  Host-side shard prep pre-transposes
the activations, regroups Wkv columns, and casts matmul operands to
bf16 (PSUM accumulation stays fp32 on device).

The mask input is structurally zero for this problem (spec fill
"zeros"); softmax(S + 0) == softmax(S), so the kernel does not load it
(checked on the host).

Per-core schedule (bf16 operands, fp32 accumulation):
  A: K^T = Wkv_K^T x_enc^T (head-pair-stacked on partitions); V in
     natural layout directly via x-stationary matmuls, next to a ones
     column that later yields the softmax denominator for free.
     Q-projection for the first q-chunk rides at the end of A.
  C: per q-chunk qc, per head pair, per k-tile: S^T = (K^T tile)^T Q^T
     with zero-padded parity copies of Q (contract over 128, two
     matmuls sharing a stationary); P^T = exp(S^T/8) on ACT; O'^T =
     V_aug^T P^T accumulated in PSUM.  The ACT engine paces this loop,
     so the q-projection of chunk qc+1 and the out-projection of chunk
     qc-1 are chopped into small matmul pieces and interleaved into
     the k-tile stream as PE fillers — PE and ACT both stay ~100%
     busy and no engine idles at chunk boundaries.
"""

import ml_dtypes
import numpy as np

import concourse.bass as bass
import concourse.mybir as mybir
import concourse.tile as tile
from concourse import bacc
from concourse.bass_utils import run_bass_kernel_spmd
from concourse.masks import make_identity

f32 = mybir.dt.float32
bf16 = mybir.dt.bfloat16
np_bf16 = ml_dtypes.bfloat16
AF = mybir.ActivationFunctionType
ALU = mybir.AluOpType

P = 128
MDT = bf16


def build_nc(T=2048, D=1024, HPC=8, HD=64, n_cores=8, debug=False):
    """Build + compile the per-core Bass program. HPC = heads per core."""
    assert HD == 64 and HPC % 2 == 0 and T % 512 == 0 and D % P == 0
    CPC = HPC * HD          # q/out channels per core
    TC = 512                # token chunk (psum free dim), phase A
    QC = 512                # q chunk, attention phase
    NQ = T // TC            # token chunks
    ND = D // P             # model-dim chunks
    NG = HPC // 2           # head pairs
    NKT = T // P            # k-token tiles
    NTT = TC // P           # token tiles per chunk (phase A V)
    HD1 = HD + 1            # V columns + ones column
    SCALE = float(1.0 / np.sqrt(HD))
    ON = min(512, D)        # out-proj free chunk
    NON = D // ON
    NQC = T // QC

    nc = bacc.Bacc("TRN2", target_bir_lowering=False, debug=False,
                   enable_asserts=False, num_devices=n_cores)

    xeT = nc.dram_tensor("x_enc_t", [D, T], MDT, kind="ExternalInput").ap()
    xdT = nc.dram_tensor("x_dec_t", [D, T], MDT, kind="ExternalInput").ap()
    wq_d = nc.dram_tensor("wq", [D, CPC], MDT, kind="ExternalInput").ap()
    # wkv is host-packed d-major ([128, ND*2*CPC]: partition p, block d
    # holds wkv row d*128+p) so the load is 2 DMAs with 8KB descriptors
    # instead of 8 with 2KB (the DGE is descriptor-rate-bound)
    wkv_d = nc.dram_tensor("wkv_g", [P, ND * 2 * CPC], MDT,
                           kind="ExternalInput").ap()
    wo_d = nc.dram_tensor("wo", [CPC, D], MDT, kind="ExternalInput").ap()
    bq_d = nc.dram_tensor("bq", [CPC], f32, kind="ExternalInput").ap()
    bkv_d = nc.dram_tensor("bkv_g", [2 * CPC], f32, kind="ExternalInput").ap()
    bo_d = nc.dram_tensor("bo", [D], MDT, kind="ExternalInput").ap()
    # out is stored qt-major: partition p, col (qt*D + c) holds
    # out[qt*128 + p, c] — adjacent qt blocks give 4KB-contiguous
    # per-partition DMA descriptors; the host un-permutes for free
    out_d = nc.dram_tensor("out", [P, (T // P) * D], MDT,
                           kind="ExternalOutput").ap()

    with tile.TileContext(nc) as tc:
      with tc.tile_pool(name="const", bufs=1) as cpool:
        # ---- tiny bias loads first, then wkv, so the first K matmul
        # ---- starts as early as possible
        bo_row = cpool.tile([1, D], MDT, name="bo_row")
        bkv_sb = cpool.tile([P, NG], f32, name="bkv_sb")
        bkv_vrow = cpool.tile([1, CPC], f32, name="bkv_vrow")
        bq_sb = cpool.tile([P, NG], f32, name="bq_sb")
        nc.sync.dma_start(out=bo_row[:], in_=bo_d[:].unsqueeze(0))
        nc.sync.dma_start(out=bkv_vrow[:],
                          in_=bkv_d[CPC:2 * CPC].unsqueeze(0))
        for g in range(NG):
            nc.sync.dma_start(out=bkv_sb[:, g:g + 1],
                              in_=bkv_d[g * P:(g + 1) * P].unsqueeze(1))
            nc.sync.dma_start(out=bq_sb[:, g:g + 1],
                              in_=bq_d[g * P:(g + 1) * P].unsqueeze(1))
        wkv_pack = cpool.tile([P, ND * 2 * CPC], MDT, name="wkv_pack")
        wkv_sb = [wkv_pack[:, d * 2 * CPC:(d + 1) * 2 * CPC]
                  for d in range(ND)]
        wq_sb = [cpool.tile([P, CPC], MDT, name=f"wq{d}") for d in range(ND)]
        wo_sb = [cpool.tile([P, D], MDT, name=f"wo{g}") for g in range(NG)]

        ident = cpool.tile([P, P], MDT, name="ident")
        make_identity(nc, ident)
        ones_t = cpool.tile([P, P], MDT, name="ones_t")
        nc.vector.tensor_scalar(ones_t[HD:HD + 1, :], ident[HD:HD + 1, :],
                                0.0, 1.0, ALU.mult, ALU.add)
        nc.vector.tensor_scalar(ones_t[0:1, :], ident[0:1, :],
                                0.0, 1.0, ALU.mult, ALU.add)

        # persistent across A->C
        kT = [cpool.tile([P, T], MDT, name=f"kT{g}") for g in range(NG)]
        vnat = [cpool.tile([P, 2 * NKT * HD1], MDT, name=f"vnat{g}")
                for g in range(NG)]     # per pair, head parity h2 in halves
        bo_bc = cpool.tile([P, D], f32, name="bo_bc")
        bkv_vbc = cpool.tile([P, CPC], f32, name="bkv_vbc")

        def vn(h):                      # per-head view [P, NKT*HD1]
            g, h2 = divmod(h, 2)
            off = h2 * NKT * HD1
            return vnat[g][:, off:off + NKT * HD1]

        # ones columns of vnat written once
        for g in range(NG):
            for h2 in range(2):
                blk = vnat[g][:, h2 * NKT * HD1:(h2 + 1) * NKT * HD1] \
                    .rearrange("p (c x) -> p c x", c=NKT)
                nc.vector.tensor_scalar(
                    blk[:, :, HD:HD1], ident[:, 0:NKT].unsqueeze(2),
                    0.0, 1.0, ALU.mult, ALU.add)

        with tc.tile_pool(name="ps_init", bufs=1, space="PSUM") as ips:
            # broadcast bo and the V-part of bkv across partitions via PE
            ps_b = ips.tile([P, D], f32, name="ps_bo")
            for o in range(0, D, 512):
                ow = min(512, D - o)
                nc.tensor.matmul(ps_b[:, o:o + ow], ones_t[0:1, :],
                                 bo_row[0:1, o:o + ow], skip_group_check=True)
            nc.vector.tensor_copy(bo_bc[:], ps_b[:])
            ps_v = ips.tile([P, CPC], f32, name="ps_bkv")
            bkv_vrow16 = cpool.tile([1, CPC], MDT, name="bkv_vrow16")
            nc.vector.tensor_copy(bkv_vrow16[:], bkv_vrow[:])
            nc.tensor.matmul(ps_v[:], ones_t[0:1, :], bkv_vrow16[0:1, :],
                             skip_group_check=True)
            nc.vector.tensor_copy(bkv_vbc[:], ps_v[:])

        # full-row activation tiles (4KB/partition DMA descriptors — the
        # DMA engines are descriptor-rate-bound, so fat rows matter) and
        # padded-parity q tiles live in the const pool
        xeT_sb = [cpool.tile([P, T], MDT, name=f"xeT{d}") for d in range(ND)]
        xdT_sb = [cpool.tile([P, T], MDT, name=f"xdT{d}") for d in range(ND)]
        qTcs = {(qc, g): cpool.tile([P, 2 * QC], MDT, tag=f"qTc{g}", bufs=2,
                                    name=f"qTc_{qc}_{g}")
                for qc in range(NQC) for g in range(NG)}

        def emit_xq_dma(qc, eng=None):
            qcols = slice(qc * QC, (qc + 1) * QC)
            e = eng if eng is not None else nc.sync
            for d in range(ND):
                e.dma_start(out=xdT_sb[d][:, qcols],
                            in_=xdT[d * P:(d + 1) * P, qcols])

        def emit_qproj(qc, g, pool, tag):
            """One filler piece: project q-chunk qc, head pair g."""
            qcols = slice(qc * QC, (qc + 1) * QC)
            pq = pool.tile([P, QC], f32, tag=tag, name=f"pq_{qc}_{g}")
            for d in range(ND):
                nc.tensor.matmul(pq[:], wq_sb[d][:, g * P:(g + 1) * P],
                                 xdT_sb[d][:, qcols],
                                 start=(d == 0), stop=(d == ND - 1),
                                 skip_group_check=True)
            qTc = qTcs[(qc, g)]
            nc.vector.tensor_scalar_add(
                qTc[0:HD, 0:QC], pq[0:HD, :], bq_sb[0:HD, g:g + 1])
            nc.vector.tensor_scalar(
                qTc[HD:P, 0:QC], pq[HD:P, :], 0.0, 0.0, ALU.mult, ALU.add)
            nc.vector.tensor_scalar_add(
                qTc[HD:P, QC:2 * QC], pq[HD:P, :], bq_sb[HD:P, g:g + 1])
            nc.vector.tensor_scalar(
                qTc[0:HD, QC:2 * QC], pq[0:HD, :], 0.0, 0.0, ALU.mult, ALU.add)

        # ---------------- Phase A: x_enc^T -> kT, vnat ----------------
        # d-outer accumulation: all 8 PSUM chains (4 K pairs + 4 V token
        # tiles) advance one d-step as soon as wkv[d] + x[d] land, so the
        # PE starts ~1us after the first DMA instead of waiting for the
        # full weight matrix.
        with tc.tile_pool(name="phA_psK", bufs=1, space="PSUM") as apsK, \
             tc.tile_pool(name="phA_psV", bufs=1, space="PSUM") as apsV:
            for tq in range(NQ - 1):     # last chunk rides inside phase C
                tcols = slice(tq * TC, (tq + 1) * TC)
                # incremental chunked loads: weights on the ACT DGE queue,
                # x_enc chunk-by-chunk on SP, x_dec chunks on ACT behind
                # the weights (all paced one token-chunk ahead of compute).
                # tq0 loads the K-half of wkv first so the K chains can
                # finish after ~2MB of DMA instead of 3MB.
                if tq == 0:
                    half = ND * CPC
                    nc.scalar.dma_start(out=wkv_pack[:, 0:half],
                                        in_=wkv_d[:, 0:half])
                    nc.scalar.dma_start(out=wkv_pack[:, half:],
                                        in_=wkv_d[:, half:])
                for d in range(ND):
                    nc.sync.dma_start(out=xeT_sb[d][:, tcols],
                                      in_=xeT[d * P:(d + 1) * P, tcols])
                if tq == 1:
                    for d in range(ND):
                        nc.scalar.dma_start(out=wq_sb[d][:],
                                            in_=wq_d[d * P:(d + 1) * P, :])
                    emit_xq_dma(0, nc.scalar)
                if tq == 2:
                    for g in range(NG):
                        nc.scalar.dma_start(out=wo_sb[g][:],
                                            in_=wo_d[g * P:(g + 1) * P, :])
                    emit_xq_dma(1, nc.scalar)
                pks = [apsK.tile([P, TC], f32, tag=f"pk{g}",
                                 name=f"pk_{tq}_{g}") for g in range(NG)]
                pvs = [apsV.tile([P, CPC], f32, tag=f"pv{tt}",
                                 name=f"pv_{tq}_{tt}") for tt in range(NTT)]
                for do_k, do_v in [(True, True)]:
                    for d in range(ND):
                        if do_k:
                            for g in range(NG):      # K groups: K^T layout
                                nc.tensor.matmul(
                                    pks[g][:],
                                    wkv_sb[d][:, g * P:(g + 1) * P],
                                    xeT_sb[d][:, tcols],
                                    start=(d == 0), stop=(d == ND - 1))
                        if do_v:
                            for tt in range(NTT):    # V natural
                                nc.tensor.matmul(
                                    pvs[tt][:],
                                    xeT_sb[d][:, tq * TC + tt * P:
                                                tq * TC + (tt + 1) * P],
                                    wkv_sb[d][:, CPC:2 * CPC],
                                    start=(d == 0), stop=(d == ND - 1))
                for g in range(NG):
                    nc.vector.tensor_scalar_add(
                        kT[g][:, tcols], pks[g][:], bkv_sb[:, g:g + 1])
                for tt in range(NTT):
                    kt_g = tq * NTT + tt     # global k-token tile index
                    for g in range(NG):      # both parities in one DVE op
                        dst = vnat[g][:, :].rearrange(
                            "p (x c y) -> p x c y", x=2, c=NKT)[:, :, kt_g, 0:HD]
                        src = pvs[tt][:, 2 * g * HD:(2 * g + 2) * HD] \
                            .rearrange("p (x y) -> p x y", x=2)
                        bias = bkv_vbc[:, 2 * g * HD:(2 * g + 2) * HD] \
                            .rearrange("p (x y) -> p x y", x=2)
                        nc.vector.tensor_add(dst, src, bias)
            # last k-chunk's x slice: loaded now, consumed by C fillers
            tcols3 = slice((NQ - 1) * TC, NQ * TC)
            for d in range(ND):
                nc.sync.dma_start(out=xeT_sb[d][:, tcols3],
                                  in_=xeT[d * P:(d + 1) * P, tcols3])
            # q-projection for the first attention chunk's g0/g1 rides
            # here; its g2/g3 ride inside chunk 0 itself
            for g in range(2):
                emit_qproj(0, g, apsV, f"pv{g}")

        def tq3_kpiece(g, pool):
            pk = pool.tile([P, TC], f32, tag="pout", name=f"pk3_{g}")
            for d in range(ND):
                nc.tensor.matmul(pk[:], wkv_sb[d][:, g * P:(g + 1) * P],
                                 xeT_sb[d][:, tcols3], start=(d == 0),
                                 stop=(d == ND - 1), skip_group_check=True)
            nc.vector.tensor_scalar_add(kT[g][:, tcols3], pk[:],
                                        bkv_sb[:, g:g + 1])

        def tq3_vpiece(tt, pool):
            kt_g = (NQ - 1) * NTT + tt
            pv = pool.tile([P, CPC], f32, tag="pout", name=f"pv3_{tt}")
            for d in range(ND):
                nc.tensor.matmul(pv[:],
                                 xeT_sb[d][:, (NQ - 1) * TC + tt * P:
                                             (NQ - 1) * TC + (tt + 1) * P],
                                 wkv_sb[d][:, CPC:2 * CPC], start=(d == 0),
                                 stop=(d == ND - 1), skip_group_check=True)
            for g in range(NG):
                dst = vnat[g][:, :].rearrange(
                    "p (x c y) -> p x c y", x=2, c=NKT)[:, :, kt_g, 0:HD]
                src = pv[:, 2 * g * HD:(2 * g + 2) * HD] \
                    .rearrange("p (x y) -> p x y", x=2)
                bias = bkv_vbc[:, 2 * g * HD:(2 * g + 2) * HD] \
                    .rearrange("p (x y) -> p x y", x=2)
                nc.vector.tensor_add(dst, src, bias)

        # ------------- Phase C: attention with interleaved fillers ----------
        with tc.tile_pool(name="phC", bufs=1) as cp2, \
             tc.tile_pool(name="phC_psS", bufs=2, space="PSUM") as psS, \
             tc.tile_pool(name="phC_psO", bufs=2, space="PSUM") as psO, \
             tc.tile_pool(name="phD_ps", bufs=2, space="PSUM") as psD:

            yTss = {}
            ost_hold = {}

            def outproj_piece(qc, qt):
                """Filler piece: one 128-row block of out = Y @ Wo + bo.
                Two qt blocks share one ost tile and leave in one fat DMA."""
                yTs = yTss[qc]
                if qt % 2 == 0:
                    ost_hold[qc] = cp2.tile([P, 2 * D], MDT, tag="ost",
                                            bufs=2, name=f"ost_{qc}_{qt}")
                ost = ost_hold[qc]
                ob = (qt % 2) * D
                for o in range(NON):
                    ocols = slice(o * ON, (o + 1) * ON)
                    pout = psD.tile([P, ON], f32, tag="pout",
                                    name=f"pout_{qc}_{qt}_{o}")
                    for g in range(NG):
                        nc.tensor.matmul(
                            pout[:], yTs[g][:, qt * P:(qt + 1) * P],
                            wo_sb[g][:, ocols],
                            start=(g == 0), stop=(g == NG - 1),
                            skip_group_check=True)
                    nc.vector.tensor_add(ost[:, ob + o * ON:ob + (o + 1) * ON],
                                         pout[:], bo_bc[:, ocols])
                if qt % 2 == 1:
                    qtg = qc * (QC // P) + qt - 1
                    # the final chunk's output drains on the by-then-idle
                    # ACT DGE queue, in parallel with the SP queue
                    eng = nc.scalar if qc == NQC - 1 and qt == 3 else nc.sync
                    eng.dma_start(
                        out=out_d[:, qtg * D:(qtg + 2) * D], in_=ost[:])

            def qp_piece(qc, g):
                return lambda: emit_qproj(qc, g, psD, "pout")

            def op_piece(qc, qt):
                return lambda: outproj_piece(qc, qt)

            for qc in range(NQC):
                # filler pieces for this chunk's k-tile stream; the last
                # K/V chunk's projections go into chunk 0 as per-head-pair
                # priority pieces (deadline: that pair's kt=12 matmul);
                # out-projections are spread over later chunks so every
                # chunk's filler supply matches its exp-pacing deficit
                gprio = {}
                if qc == 0:
                    # vpieces feed every pair's kt=12..15 -> all inside g0;
                    # kpiece g feeds only pair g's kt=12..15 -> one per g
                    gprio[0] = [lambda tt=tt: tq3_vpiece(tt, psD)
                                for tt in range(NTT)] \
                        + [lambda: tq3_kpiece(0, psD)]
                    gprio[1] = [lambda: tq3_kpiece(1, psD), qp_piece(0, 2)]
                    gprio[2] = [lambda: tq3_kpiece(2, psD), qp_piece(0, 3)]
                    gprio[3] = [lambda: tq3_kpiece(3, psD)]
                    rest = [lambda: emit_xq_dma(2)] \
                        + [qp_piece(1, 0), qp_piece(1, 1)]
                else:
                    # this chunk's own g2/g3 q-projections ride in its g0/g1
                    gprio[0] = [qp_piece(qc, 2)]
                    gprio[1] = [qp_piece(qc, 3)]
                    if qc == 1:
                        rest = [lambda: emit_xq_dma(3)] \
                            + [qp_piece(2, 0), qp_piece(2, 1),
                               op_piece(0, 0), op_piece(0, 1)]
                    elif qc == 2:
                        rest = [qp_piece(3, 0), qp_piece(3, 1),
                                op_piece(0, 2), op_piece(0, 3),
                                op_piece(1, 0)]
                    else:
                        rest = [op_piece(1, 1), op_piece(1, 2),
                                op_piece(1, 3)] \
                            + [op_piece(2, qt) for qt in range(QC // P)]
                nslots = NG * (NKT // 2)
                nrest = len(rest)
                rpopped = 0
                slot = 0
                cur_prio = []

                def tick():
                    nonlocal slot, rpopped
                    slot += 1
                    if cur_prio:
                        cur_prio.pop(0)()
                        return
                    while rpopped < min(nrest, nrest * slot // nslots):
                        rest[rpopped]()
                        rpopped += 1

                yTs = [cp2.tile([P, QC], MDT, tag=f"yT{g}", bufs=3,
                                name=f"yT_{qc}_{g}") for g in range(NG)]
                yTss[qc] = yTs
                for g in range(NG):
                    cur_prio = gprio.get(g, [])
                    po = [psO.tile([P, QC], f32, tag="po",
                                   name=f"po_{qc}_{g}_{h2}")
                          for h2 in range(2)]
                    pss = {}
                    pts = {}
                    qTc = qTcs[(qc, g)]

                    def mm1(kt, qc=qc, g=g, qTc=qTc, pss=pss):
                        ps = psS.tile([P, 2 * QC], f32, tag="ps",
                                      name=f"ps_{qc}_{g}_{kt}")
                        for h2 in range(2):
                            nc.tensor.matmul(
                                ps[:, h2 * QC:(h2 + 1) * QC],
                                kT[g][:, kt * P:(kt + 1) * P],
                                qTc[:, h2 * QC:(h2 + 1) * QC],
                                skip_group_check=True)
                        pss[kt] = ps

                    def do_exp(kt, qc=qc, g=g, pss=pss, pts=pts):
                        pt = cp2.tile([P, 2 * QC], MDT, tag="pt", bufs=4,
                                      name=f"pt_{qc}_{g}_{kt}")
                        nc.scalar.activation(pt[:], pss[kt][:], AF.Exp,
                                             scale=SCALE)
                        pts[kt] = pt

                    def mm2(kt, g=g, po=po, pts=pts):
                        pt = pts[kt]
                        for h2 in range(2):
                            nc.tensor.matmul(
                                po[h2][0:HD1, :],
                                vn(2 * g + h2)[:, kt * HD1:(kt + 1) * HD1],
                                pt[:, h2 * QC:(h2 + 1) * QC],
                                start=(kt == 0), stop=(kt == NKT - 1),
                                skip_group_check=True)

                    # software pipeline over kt with PE fillers
                    mm1(0)
                    mm1(1)
                    do_exp(0)
                    do_exp(1)
                    for k2 in range(1, NKT // 2):
                        mm1(2 * k2)
                        mm1(2 * k2 + 1)
                        mm2(2 * k2 - 2)
                        mm2(2 * k2 - 1)
                        do_exp(2 * k2)
                        do_exp(2 * k2 + 1)
                        tick()
                    mm2(NKT - 2)
                    mm2(NKT - 1)
                    tick()

                    # normalize + evict to yT
                    srow = [cp2.tile([P, QC], MDT, tag="srow", bufs=2,
                                     name=f"srow_{qc}_{g}_{h2}")
                            for h2 in range(2)]
                    rbc = [cp2.tile([P, QC], f32, tag="rbc", bufs=2,
                                    name=f"rbc_{qc}_{g}_{h2}")
                           for h2 in range(2)]
                    scr = [cp2.tile([P, QC], f32, tag="scr", bufs=2,
                                    name=f"scr_{qc}_{g}_{h2}")
                           for h2 in range(2)]
                    ps_bc = [psD.tile([P, QC], f32, tag="pout",
                                      name=f"psbc_{qc}_{g}_{h2}")
                             for h2 in range(2)]
                    for h2 in range(2):
                        nc.vector.tensor_copy(srow[h2][HD:HD1, :],
                                              po[h2][HD:HD1, :])
                    for h2 in range(2):
                        nc.tensor.matmul(ps_bc[h2][0:HD, :],
                                         ones_t[HD:HD1, 0:HD],
                                         srow[h2][HD:HD1, :],
                                         skip_group_check=True)
                    for h2 in range(2):
                        nc.vector.reciprocal_approx_accurate(
                            out=rbc[h2][0:HD, :], in_=ps_bc[h2][0:HD, :],
                            scratch=scr[h2][0:HD, :])
                    nc.vector.tensor_mul(yTs[g][0:HD, :],
                                         po[0][0:HD, :], rbc[0][0:HD, :])
                    stg = cp2.tile([P, QC], MDT, tag="stg",
                                   bufs=2, name=f"stg_{qc}_{g}")
                    nc.vector.tensor_mul(stg[0:HD, :],
                                         po[1][0:HD, :], rbc[1][0:HD, :])
                    nc.sync.dma_start(out=yTs[g][HD:P, :],
                                      in_=stg[0:HD, :])
                # leftovers (shouldn't happen, but keep it correct)
                for g in range(NG):
                    while gprio.get(g):
                        gprio[g].pop(0)()
                while rpopped < nrest:
                    rest[rpopped]()
                    rpopped += 1

            # tail: out-projection of the last chunk
            for qt in range(QC // P):
                outproj_piece(NQC - 1, qt)

    nc.compile()
    return nc


# ---------------------------------------------------------------------------
# Host side: sharding, run, unshard
# ---------------------------------------------------------------------------

_NC_CACHE = {}


def _get_nc():
    key = "full"
    if key not in _NC_CACHE:
        _NC_CACHE[key] = build_nc()
    return _NC_CACHE[key]


def _group_kv_cols(w_slice, HPC, HD):
    """Reorder kv columns [h-major, (k|v), d] -> K head-pair groups then V."""
    last = w_slice.shape[-1]
    assert last == HPC * 2 * HD
    arr = w_slice.reshape(w_slice.shape[:-1] + (HPC, 2, HD))
    kpart = arr[..., :, 0, :].reshape(w_slice.shape[:-1] + (HPC * HD,))
    vpart = arr[..., :, 1, :].reshape(w_slice.shape[:-1] + (HPC * HD,))
    return np.ascontiguousarray(np.concatenate([kpart, vpart], axis=-1))


def make_in_maps(x_enc, x_dec, Wq, bq, Wkv, bkv, Wo, bo, n_cores=8,
                 HPC=8, HD=64):
    CPC = HPC * HD
    mdt = np_bf16
    in_maps = []
    xet = [np.ascontiguousarray(x_enc[b].T).astype(mdt)
           for b in range(x_enc.shape[0])]
    xdt = [np.ascontiguousarray(x_dec[b].T).astype(mdt)
           for b in range(x_dec.shape[0])]
    for c in range(n_cores):
        b, hg = c // 2, c % 2
        wkv_slice = Wkv[:, hg * 2 * CPC:(hg + 1) * 2 * CPC]
        bkv_slice = bkv[hg * 2 * CPC:(hg + 1) * 2 * CPC]
        in_maps.append({
            "x_enc_t": xet[b],
            "x_dec_t": xdt[b],
            "wq": np.ascontiguousarray(Wq[:, hg * CPC:(hg + 1) * CPC]).astype(mdt),
            "wkv_g": np.ascontiguousarray(
                _group_kv_cols(wkv_slice, HPC, HD).astype(mdt)
                .reshape(8, 128, 2 * CPC).transpose(1, 0, 2)
                .reshape(128, 8 * 2 * CPC)),
            "wo": np.ascontiguousarray(Wo[hg * CPC:(hg + 1) * CPC, :]).astype(mdt),
            "bq": np.ascontiguousarray(bq[hg * CPC:(hg + 1) * CPC]),
            "bkv_g": _group_kv_cols(bkv_slice, HPC, HD),
            "bo": (np.ascontiguousarray(bo) if hg == 0
                   else np.zeros_like(bo)).astype(mdt),
        })
    return in_maps


def unshard(outs, B=4, T=2048, D=1024):
    def unpack(a):
        # device layout: [128, (T//128)*D] qt-major -> [T, D]
        return np.ascontiguousarray(
            a.astype(np.float32).reshape(P, T // P, D).transpose(1, 0, 2)
            .reshape(T, D))
    return np.stack([unpack(outs[2 * b]) + unpack(outs[2 * b + 1])
                     for b in range(B)], axis=0)


def kernel(x_enc, x_dec, mask, Wq, bq, Wkv, bkv, Wo, bo):
    x_enc = np.asarray(x_enc, dtype=np.float32)
    x_dec = np.asarray(x_dec, dtype=np.float32)
    Wq = np.asarray(Wq, dtype=np.float32)
    bq = np.asarray(bq, dtype=np.float32)
    Wkv = np.asarray(Wkv, dtype=np.float32)
    bkv = np.asarray(bkv, dtype=np.float32)
    Wo = np.asarray(Wo, dtype=np.float32)
    bo = np.asarray(bo, dtype=np.float32)
    mask = np.asarray(mask)
    if mask.any():
        raise ValueError("kernel assumes a zero additive mask (spec fill=zeros)")

    nc = _get_nc()
    in_maps = make_in_maps(x_enc, x_dec, Wq, bq, Wkv, bkv, Wo, bo)
    res = run_bass_kernel_spmd(nc, in_maps, core_ids=list(range(8)))
    outs = [res.results[c]["out"] for c in range(8)]
    return unshard(outs, B=x_enc.shape[0])


if __name__ == "__main__":
    import time
    t0 = time.time()
    nc = _get_nc()
    print(f"build+compile ok in {time.time() - t0:.1f}s")
